# revision 15
# baseline (speedup 1.0000x reference)
"""Trainium2 Bass kernel for nn_Attention_24704651887034.

Dense ViT-style attention block (B=16, N=1024, C=768, H=12 heads, 2D RoPE),
data-parallel over batch across 8 NeuronCores (2 batch items per core, no
collectives).

Per-core device kernel (all matmuls bf16, fp32 PSUM accumulation). All PSUM
accumulation groups run through [128,1024] 2-bank tiles so one stationary
load covers 1024 moving columns (walrus runs with ldw-opt disabled, so every
stationary switch costs ~110ns serially). Softmax normalization broadcasts
1/denom rows across partitions with a ones-outer-product matmul into PSUM
(a DMA row-broadcast costs ~11us; the matmul costs ~0.4us).

  Phase A: qk feature-major matmul + fused 2D RoPE and v token-major (with
           per-head [v_h | ones] column so attnv PSUM row 64 accumulates
           softmax denominators) for BOTH items; PSUM->SBUF copies on the
           otherwise-idle Scalar/ACT engine.
  Phase B: attention item0 (per head: sT=k'q'^T, Exp on ACT, outT=[v|1]pT),
           PSUM drains on DVE; per-head denominator row gathered to sums0.
  Phase C: attention item1, interleaved with item0's normalize (reciprocal
           on DVE + matmul row-broadcast + in-place ao multiply).
  Phase D: proj item0, item1 normalize, proj item1; proj copies on ACT.
"""

import numpy as np

_B, _N, _C, _H = 16, 1024, 768, 12
_HD, _DR = 64, 32
_ROPE_BASE = 10000.0
_NCORES = 8
_BL = _B // _NCORES  # batch items per core

_NC6 = _C // 128      # 6 contraction chunks
_NF12 = 2 * _C // 128  # 12 feature chunks for q,k

_nc_cache = {}


def _split_excess_waits(nc, max_waits=1):
    """Walrus in this toolchain accepts at most one sync-wait command per
    instruction; Tile's tail drain (and occasionally the scheduler) emits
    more. Split the excess onto same-engine NOPs inserted just before."""
    from concourse import mybir

    for f in nc.m.functions:
        for blk in f.blocks:
            insts = blk.instructions
            i = 0
            while i < len(insts):
                ins = insts[i]
                si = getattr(ins, "sync_info", None)
                if si is not None and len(si.on_wait) > max_waits:
                    excess = si.on_wait[max_waits:]
                    ins.sync_info = mybir.SyncInfo(
                        on_wait=list(si.on_wait[:max_waits]),
                        on_update=list(si.on_update),
                    )
                    for j, w in enumerate(excess):
                        nop = mybir.InstNoOp(
                            name=f"{ins.name}-sw{j}", engine=ins.engine
                        )
                        nop.sync_info = mybir.SyncInfo(on_wait=[w], on_update=[])
                        insts.insert(i, nop)
                        i += 1
                i += 1
    return nc


def _build(has_bias):
    from contextlib import ExitStack

    import concourse.bass as bass
    import concourse.tile as tile
    from concourse import mybir

    BF = mybir.dt.bfloat16
    F32 = mybir.dt.float32
    Exp = mybir.ActivationFunctionType.Exp
    N, C, H = _N, _C, _H
    BL = _BL

    nc = bass.Bass("TRN2", target_bir_lowering=False, debug=False)
    x_d = nc.dram_tensor("xt", [BL * C, N], BF, kind="ExternalInput").ap()
    wq_d = nc.dram_tensor("wq", [C, 3 * C], BF, kind="ExternalInput").ap()
    wp_d = nc.dram_tensor("wp", [C, C], BF, kind="ExternalInput").ap()
    cos_d = nc.dram_tensor("cosm", [BL * 128, N], BF, kind="ExternalInput").ap()
    sin_d = nc.dram_tensor("ssinm", [BL * 128, N], BF, kind="ExternalInput").ap()
    if has_bias:
        bqk_d = nc.dram_tensor("bqk", [1, 2 * C], BF, kind="ExternalInput").ap()
        bv_d = nc.dram_tensor("bv", [1, C], BF, kind="ExternalInput").ap()
        bp_d = nc.dram_tensor("bp", [1, C], BF, kind="ExternalInput").ap()
    out_d = nc.dram_tensor("out", [BL * N, C], BF, kind="ExternalOutput").ap()

    SH_MASK = [(i + 16) % 32 for i in range(32)]

    with tile.TileContext(nc) as tc, ExitStack() as ctx:
        const = ctx.enter_context(tc.tile_pool(name="const", bufs=1))
        xT_p = ctx.enter_context(tc.tile_pool(name="xT", bufs=12))
        map_p = ctx.enter_context(tc.tile_pool(name="maps", bufs=2))
        raw_p = ctx.enter_context(tc.tile_pool(name="qkraw", bufs=2))
        tmp_p = ctx.enter_context(tc.tile_pool(name="tmp", bufs=3))
        qkr_p = ctx.enter_context(tc.tile_pool(name="qkr", bufs=24))
        v_p = ctx.enter_context(tc.tile_pool(name="v", bufs=16))
        pT_p = ctx.enter_context(tc.tile_pool(name="pT", bufs=3))
        ao_p = ctx.enter_context(tc.tile_pool(name="ao", bufs=12))
        sum_p = ctx.enter_context(tc.tile_pool(name="sums", bufs=2))
        ost_p = ctx.enter_context(tc.tile_pool(name="ost", bufs=2))
        sc_ps = ctx.enter_context(tc.tile_pool(name="scps", bufs=2, space="PSUM"))
        bc_ps = ctx.enter_context(tc.tile_pool(name="bcps", bufs=1, space="PSUM"))
        o_ps = ctx.enter_context(tc.tile_pool(name="ops", bufs=2, space="PSUM"))

        # ---- resident constants ----
        ones64 = const.tile([1, 64], BF, tag="ones64", name="ones64")
        nc.gpsimd.memset(ones64[:], 1.0)
        wq_t = []
        for c in range(_NC6):
            t = const.tile([128, 3 * C], BF, tag=f"wq{c}", name=f"wq{c}")
            nc.sync.dma_start(
                t[:, 0:2 * C], wq_d[c * 128:(c + 1) * 128, 0:2 * C]
            )
            wq_t.append(t)
        if has_bias:
            bqk_sb = const.tile([1, 2 * C], BF, tag="bqk")
            nc.sync.dma_start(bqk_sb[:], bqk_d[:])
            bv_sb = const.tile([1, C], BF, tag="bv")
            nc.sync.dma_start(bv_sb[:], bv_d[:])
            bp_sb = const.tile([1, C], BF, tag="bp")
            nc.sync.dma_start(bp_sb[:], bp_d[:])
            ones_r = const.tile([1, 512], BF, tag="ones")
            nc.gpsimd.memset(ones_r[:], 1.0)

        # ---- inputs for both items up front (DMA overlaps compute) ----
        xT = []
        cosm = []
        ssin = []
        for b in range(BL):
            xTb = []
            for c in range(_NC6):
                t = xT_p.tile([128, N], BF, tag="xT", name=f"xT{b}_{c}")
                nc.sync.dma_start(
                    t[:], x_d[b * C + c * 128:b * C + (c + 1) * 128, :]
                )
                xTb.append(t)
            xT.append(xTb)
            cm = map_p.tile([128, N], BF, tag="cos", name=f"cos{b}")
            nc.sync.dma_start(cm[:], cos_d[b * 128:(b + 1) * 128, :])
            cosm.append(cm)
            sm = map_p.tile([128, N], BF, tag="sin", name=f"sin{b}")
            nc.sync.dma_start(sm[:], sin_d[b * 128:(b + 1) * 128, :])
            ssin.append(sm)

        # v-columns of wq and wp arrive after the hot qk inputs
        for c in range(_NC6):
            nc.sync.dma_start(
                wq_t[c][:, 2 * C:3 * C], wq_d[c * 128:(c + 1) * 128, 2 * C:3 * C]
            )
        wp_t = []
        for c in range(_NC6):
            t = const.tile([128, C], BF, tag=f"wp{c}", name=f"wp{c}")
            nc.sync.dma_start(t[:], wp_d[c * 128:(c + 1) * 128, :])
            wp_t.append(t)

        _grp = [0]

        def acc_tile(name):
            """Rotate accumulation groups across 3 PSUM tiles (sc ring of 2
            plus the broadcast bank) for 3-deep group pipelining."""
            _grp[0] += 1
            if _grp[0] % 3 == 0:
                return bc_ps.tile([128, N], F32, tag="bcp", name=name)
            return sc_ps.tile([128, N], F32, tag="sc", name=name)

        def emit_qk_feature(b, f):
            """q/k feature chunk: one stationary load per contraction chunk
            covers both 512-col halves of a 2-bank PSUM tile; fused RoPE."""
            raw = raw_p.tile([128, N], BF, tag="qkraw", name=f"raw{b}_{f}")
            ps = acc_tile(f"mq{b}_{f}")
            for c in range(_NC6):
                for t2 in range(2):
                    nc.tensor.matmul(
                        ps[:, t2 * 512:(t2 + 1) * 512],
                        wq_t[c][:, f * 128:(f + 1) * 128],
                        xT[b][c][:, t2 * 512:(t2 + 1) * 512],
                        start=(c == 0),
                        stop=(c == _NC6 - 1 and not has_bias),
                    )
            if has_bias:
                for t2 in range(2):
                    nc.tensor.matmul(
                        ps[:, t2 * 512:(t2 + 1) * 512],
                        bqk_sb[:, f * 128:(f + 1) * 128],
                        ones_r[:],
                        start=False,
                        stop=True,
                    )
            nc.scalar.copy(raw[:], ps[:])
            r = tmp_p.tile([128, N], BF, tag="ttmp", name=f"r{b}_{f}")
            nc.vector.stream_shuffle(r[:], raw[:], SH_MASK)
            tm = tmp_p.tile([128, N], BF, tag="ttmp", name=f"tm{b}_{f}")
            nc.vector.tensor_mul(tm[:], r[:], ssin[b][:])
            am = tmp_p.tile([128, N], BF, tag="ttmp", name=f"am{b}_{f}")
            nc.gpsimd.tensor_mul(am[:], raw[:], cosm[b][:])
            ro = qkr_p.tile([128, N], BF, tag="qkr", name=f"qkr{b}_{f}")
            nc.vector.tensor_add(ro[:], tm[:], am[:])
            return ro

        def emit_v(b, t8):
            """v token-major, per-head [v_h | ones] layout; one stationary
            (x chunk) load per contraction chunk covers all 768 v columns."""
            vt = v_p.tile([128, H * 65], BF, tag="v", name=f"v{b}_{t8}")
            vt3 = vt.rearrange("p (h w) -> p h w", w=65)
            nc.gpsimd.memset(vt3[:, :, 64:65], 1.0)
            ps = acc_tile(f"mv{b}_{t8}")
            for c in range(_NC6):
                for f0, fw in ((0, 512), (512, 256)):
                    nc.tensor.matmul(
                        ps[:, f0:f0 + fw],
                        xT[b][c][:, t8 * 128:(t8 + 1) * 128],
                        wq_t[c][:, 2 * C + f0:2 * C + f0 + fw],
                        start=(c == 0),
                        stop=(c == _NC6 - 1 and not has_bias),
                    )
            if has_bias:
                for f0, fw in ((0, 512), (512, 256)):
                    nc.tensor.matmul(
                        ps[:, f0:f0 + fw],
                        ones_r[:, 0:128],
                        bv_sb[:, f0:f0 + fw],
                        start=False,
                        stop=True,
                    )
            nc.scalar.copy(
                vt3[:, :, 0:64],
                ps[:, 0:768].rearrange("p (h w) -> p h w", w=64),
            )
            return vt

        def emit_head(b, h, qk_r, v_sb, ao_t, sums_st):
            """One attention head: scores -> exp -> attnv, k-chunk streamed.
            Denominator row (PSUM row 64) and ao rows drain on DVE."""
            jj, half = h // 2, (h % 2) * 64
            qh = qk_r[jj]
            kh = qk_r[6 + jj]
            op = [
                o_ps.tile([65, 512], F32, tag="o", name=f"op{b}_{h}_{i}")
                for i in range(2)
            ]
            pT_tiles = [None] * 8
            for kc in range(9):
                if kc < 8:
                    pT = pT_p.tile([128, N], BF, tag="pT", name=f"p{b}_{h}_{kc}")
                    s = sc_ps.tile([128, N], F32, tag="sc", name=f"s{b}_{h}_{kc}")
                    for qc in range(2):
                        nc.tensor.matmul(
                            s[:, qc * 512:(qc + 1) * 512],
                            kh[half:half + 64, kc * 128:(kc + 1) * 128],
                            qh[half:half + 64, qc * 512:(qc + 1) * 512],
                            start=True,
                            stop=True,
                        )
                    nc.scalar.activation(pT[:], s[:], Exp, scale=0.125)
                    pT_tiles[kc] = pT
                if kc >= 1:
                    kd = kc - 1
                    for qc in range(2):
                        nc.tensor.matmul(
                            op[qc][:],
                            v_sb[kd][:, h * 65:(h + 1) * 65],
                            pT_tiles[kd][:, qc * 512:(qc + 1) * 512],
                            start=(kd == 0),
                            stop=(kd == 7),
                        )
            srow = sum_p.tile([1, N], BF, tag="srow", bufs=1, name=f"sr{b}_{h}")
            for qc in range(2):
                nc.vector.tensor_copy(
                    srow[:, qc * 512:(qc + 1) * 512], op[qc][64:65, :]
                )
                nc.vector.tensor_copy(
                    ao_t[jj][half:half + 64, qc * 512:(qc + 1) * 512],
                    op[qc][0:64, :],
                )
            nc.gpsimd.dma_start(
                sums_st[8 * h:8 * (h + 1), :],
                srow[:].rearrange("p (a n) -> p a n", a=8),
            )

        def emit_recip(b, sums_st):
            """1/denominators for all 12 heads; each row is DMA'd back to a
            partition-0 tile so the broadcast matmul can use it as a moving
            operand (engine/matmul partition bases must be 0/32/64/96)."""
            sumf = sum_p.tile(
                [8 * H, N // 8], F32, tag="sumf", bufs=1, name=f"sf{b}"
            )
            nc.vector.tensor_copy(sumf[:], sums_st[:])
            recipf = sum_p.tile(
                [8 * H, N // 8], F32, tag="recipf", bufs=1, name=f"rf{b}"
            )
            nc.vector.reciprocal(recipf[:], sumf[:])
            recipb = sum_p.tile(
                [8 * H, N // 8], BF, tag="recipb", bufs=1, name=f"rb{b}"
            )
            nc.vector.tensor_copy(recipb[:], recipf[:])
            rows = []
            for h in range(H):
                rr = sum_p.tile(
                    [1, N], BF, tag="rbrow", bufs=4, name=f"rr{b}_{h}"
                )
                nc.gpsimd.dma_start(
                    rr[:].rearrange("p (a n) -> p a n", a=8),
                    recipb[8 * h:8 * (h + 1), :].rearrange(
                        "p (u n) -> p u n", u=1
                    ),
                )
                rows.append(rr)
            return rows

        def emit_norm(b, jj, recipb, ao_t):
            """Broadcast 1/denom rows for heads 2jj,2jj+1 across 64
            partitions each via ones-outer-product matmuls, then normalize
            the ao tile in place."""
            ps = bc_ps.tile([128, N], F32, tag="bcp", name=f"bcp{b}_{jj}")
            for k in range(2):
                for qc in range(2):
                    nc.tensor.matmul(
                        ps[k * 64:(k + 1) * 64, qc * 512:(qc + 1) * 512],
                        ones64[:],
                        recipb[2 * jj + k][:, qc * 512:(qc + 1) * 512],
                        start=True,
                        stop=True,
                    )
            nc.vector.tensor_mul(ao_t[jj][:], ao_t[jj][:], ps[:])

        def emit_proj(b, t8, ao_t, on_vec):
            """out[t8] = ao @ W_proj; one stationary (ao chunk) load per
            contraction chunk covers both 384-col output halves."""
            copy = nc.vector.tensor_copy if on_vec else nc.scalar.copy
            ot = ost_p.tile([128, C], BF, tag="ost", name=f"ost{b}_{t8}")
            ps = acc_tile(f"mp{b}_{t8}")
            for c in range(_NC6):
                for nf in range(2):
                    nc.tensor.matmul(
                        ps[:, nf * 512:nf * 512 + 384],
                        ao_t[c][:, t8 * 128:(t8 + 1) * 128],
                        wp_t[c][:, nf * 384:(nf + 1) * 384],
                        start=(c == 0),
                        stop=(c == _NC6 - 1 and not has_bias),
                    )
            if has_bias:
                for nf in range(2):
                    nc.tensor.matmul(
                        ps[:, nf * 512:nf * 512 + 384],
                        ones_r[:, 0:128],
                        bp_sb[:, nf * 384:(nf + 1) * 384],
                        start=False,
                        stop=True,
                    )
            for nf in range(2):
                copy(ot[:, nf * 384:(nf + 1) * 384],
                     ps[:, nf * 512:nf * 512 + 384])
            nc.sync.dma_start(
                out_d[b * N + t8 * 128:b * N + (t8 + 1) * 128, :], ot[:]
            )

        # ---- Phase A: qkv for both items (copies on idle ACT) ----
        qk_r0 = [emit_qk_feature(0, f) for f in range(_NF12)]
        v0 = [emit_v(0, t8) for t8 in range(8)]
        qk_r1 = [emit_qk_feature(1, f) for f in range(_NF12)]
        v1 = [emit_v(1, t8) for t8 in range(8)]

        # ---- Phase B: attention item0 ----
        sums0 = sum_p.tile([8 * H, N // 8], BF, tag="sums", name="sums0")
        ao0 = [
            ao_p.tile([128, N], BF, tag="ao", name=f"ao0_{i}")
            for i in range(_NC6)
        ]
        for h in range(H):
            emit_head(0, h, qk_r0, v0, ao0, sums0)
        recipb0 = emit_recip(0, sums0)

        # ---- Phase C: attention item1; item0 normalize then proj0 ----
        sums1 = sum_p.tile([8 * H, N // 8], BF, tag="sums", name="sums1")
        ao1 = [
            ao_p.tile([128, N], BF, tag="ao", name=f"ao1_{i}")
            for i in range(_NC6)
        ]
        for h in range(H):
            emit_head(1, h, qk_r1, v1, ao1, sums1)
            if 1 <= h <= 6:
                emit_norm(0, h - 1, recipb0, ao0)
        recipb1 = emit_recip(1, sums1)

        # ---- Phase D: proj item0, item1 normalize, proj item1 ----
        for t8 in range(4):
            emit_proj(0, t8, ao0, on_vec=(t8 % 2 == 1))
        for jj in range(_NC6):
            emit_norm(1, jj, recipb1, ao1)
        for t8 in range(4, 8):
            emit_proj(0, t8, ao0, on_vec=(t8 % 2 == 1))
        for t8 in range(8):
            emit_proj(1, t8, ao1, on_vec=(t8 % 2 == 1))
    return _split_excess_waits(nc)


def _get_nc(has_bias):
    if has_bias not in _nc_cache:
        _nc_cache[has_bias] = _build(has_bias)
    return _nc_cache[has_bias]


def _prep_in_maps(x, W_qkv, b_qkv, W_proj, b_proj, pos_h, pos_w):
    import ml_dtypes

    bf16 = ml_dtypes.bfloat16
    has_bias = bool(np.any(b_qkv)) or bool(np.any(b_proj))

    inv = 1.0 / _ROPE_BASE ** (
        np.arange(0, _DR, 2, dtype=np.float32) / float(_DR)
    )  # [16]

    def rope_maps(pos):
        ang = pos.astype(np.float32)[..., None] * inv  # [B, N, 16]
        cos = np.repeat(np.cos(ang), 2, axis=-1)  # [B, N, 32]
        sin = np.repeat(np.sin(ang), 2, axis=-1)
        return cos.transpose(0, 2, 1), sin.transpose(0, 2, 1)  # [B, 32, N]

    ch, sh = rope_maps(np.asarray(pos_h))
    cw, sw = rope_maps(np.asarray(pos_w))
    cos64 = np.concatenate([ch, cw], axis=1)  # [B, 64, N]
    sin64 = np.concatenate([sh, sw], axis=1)
    sign = np.where((np.arange(64) % 32) < 16, -1.0, 1.0).astype(np.float32)
    ssin64 = sin64 * sign[None, :, None]
    cosm = np.tile(cos64, (1, 2, 1)).astype(bf16)  # [B, 128, N]
    ssinm = np.tile(ssin64, (1, 2, 1)).astype(bf16)

    xb = np.asarray(x).astype(bf16)
    xtb = xb.transpose(0, 2, 1)  # [B, C, N] feature-major per item
    wqb = np.ascontiguousarray(np.asarray(W_qkv).astype(bf16))
    wpb = np.ascontiguousarray(np.asarray(W_proj).astype(bf16))

    in_maps = []
    for i in range(_NCORES):
        lo, hi = i * _BL, (i + 1) * _BL
        m = {
            "xt": np.ascontiguousarray(xtb[lo:hi].reshape(_BL * _C, _N)),
            "wq": wqb,
            "wp": wpb,
            "cosm": np.ascontiguousarray(cosm[lo:hi].reshape(_BL * 128, _N)),
            "ssinm": np.ascontiguousarray(ssinm[lo:hi].reshape(_BL * 128, _N)),
        }
        if has_bias:
            bq = np.asarray(b_qkv).astype(bf16)
            m["bqk"] = np.ascontiguousarray(bq[:2 * _C].reshape(1, 2 * _C))
            m["bv"] = np.ascontiguousarray(bq[2 * _C:].reshape(1, _C))
            m["bp"] = np.ascontiguousarray(
                np.asarray(b_proj).astype(bf16).reshape(1, _C)
            )
        in_maps.append(m)
    return in_maps, has_bias


def _ensure_ntff_hook():
    """This image's antenv lacks axon_hooks; recreate it from the boot
    helper so run_bass_kernel_spmd(trace=True) can capture NTFF profiles."""
    import sys
    import types

    if "antenv.axon_hooks" in sys.modules:
        return
    try:
        from trn_agent_boot.trn_boot import _ntff_profile_via_ctypes

        hook = _ntff_profile_via_ctypes("/opt/axon/libaxon_pjrt.so")
    except Exception:
        hook = None
    mod = types.ModuleType("antenv.axon_hooks")
    mod._hook = hook
    mod.get_axon_ntff_profile_hook = lambda: mod._hook
    mod.set_axon_ntff_profile_hook = lambda h: setattr(mod, "_hook", h)
    sys.modules["antenv.axon_hooks"] = mod


def run(x, W_qkv, b_qkv, W_proj, b_proj, pos_h, pos_w, num_heads, **run_kwargs):
    """Build + execute on 8 NeuronCores; returns (output, BassKernelResults)."""
    from concourse.bass_utils import run_bass_kernel_spmd

    if run_kwargs.get("trace"):
        _ensure_ntff_hook()

    assert int(num_heads) == _H
    in_maps, has_bias = _prep_in_maps(
        x, W_qkv, b_qkv, W_proj, b_proj, pos_h, pos_w
    )
    nc = _get_nc(has_bias)
    res = run_bass_kernel_spmd(
        nc, in_maps, core_ids=list(range(_NCORES)), **run_kwargs
    )
    out = np.concatenate(
        [res.results[i]["out"].reshape(_BL, _N, _C) for i in range(_NCORES)],
        axis=0,
    ).astype(np.float32)
    return out, res


def kernel(x, W_qkv, b_qkv, W_proj, b_proj, pos_h, pos_w, num_heads):
    out, _ = run(x, W_qkv, b_qkv, W_proj, b_proj, pos_h, pos_w, num_heads)
    return out


# revision 16
# speedup vs baseline: 1.0280x; 1.0280x over previous
"""Trainium2 Bass kernel for nn_Attention_24704651887034.

Dense ViT-style attention block (B=16, N=1024, C=768, H=12 heads, 2D RoPE),
data-parallel over batch across 8 NeuronCores (2 batch items per core, no
collectives).

Per-core device kernel (all matmuls bf16, fp32 PSUM accumulation). All PSUM
accumulation groups run through [128,1024] 2-bank tiles so one stationary
load covers 1024 moving columns (walrus runs with ldw-opt disabled, so every
stationary switch costs ~110ns serially). Softmax normalization broadcasts
1/denom rows across partitions with a ones-outer-product matmul into PSUM
(a DMA row-broadcast costs ~11us; the matmul costs ~0.4us).

  Phase A: qk feature-major matmul + fused 2D RoPE and v token-major (with
           per-head [v_h | ones] column so attnv PSUM row 64 accumulates
           softmax denominators) for BOTH items; PSUM->SBUF copies on the
           otherwise-idle Scalar/ACT engine.
  Phase B: attention item0 (per head: sT=k'q'^T, Exp on ACT, outT=[v|1]pT),
           PSUM drains on DVE; per-head denominator row gathered to sums0.
  Phase C: attention item1, interleaved with item0's normalize (reciprocal
           on DVE + matmul row-broadcast + in-place ao multiply).
  Phase D: proj item0, item1 normalize, proj item1; proj copies on ACT.
"""

import numpy as np

_B, _N, _C, _H = 16, 1024, 768, 12
_HD, _DR = 64, 32
_ROPE_BASE = 10000.0
_NCORES = 8
_BL = _B // _NCORES  # batch items per core

_NC6 = _C // 128      # 6 contraction chunks
_NF12 = 2 * _C // 128  # 12 feature chunks for q,k

_nc_cache = {}


def _split_excess_waits(nc, max_waits=1):
    """Walrus in this toolchain accepts at most one sync-wait command per
    instruction; Tile's tail drain (and occasionally the scheduler) emits
    more. Split the excess onto same-engine NOPs inserted just before."""
    from concourse import mybir

    for f in nc.m.functions:
        for blk in f.blocks:
            insts = blk.instructions
            i = 0
            while i < len(insts):
                ins = insts[i]
                si = getattr(ins, "sync_info", None)
                if si is not None and len(si.on_wait) > max_waits:
                    excess = si.on_wait[max_waits:]
                    ins.sync_info = mybir.SyncInfo(
                        on_wait=list(si.on_wait[:max_waits]),
                        on_update=list(si.on_update),
                    )
                    for j, w in enumerate(excess):
                        nop = mybir.InstNoOp(
                            name=f"{ins.name}-sw{j}", engine=ins.engine
                        )
                        nop.sync_info = mybir.SyncInfo(on_wait=[w], on_update=[])
                        insts.insert(i, nop)
                        i += 1
                i += 1
    return nc


def _build(has_bias):
    from contextlib import ExitStack

    import concourse.bass as bass
    import concourse.tile as tile
    from concourse import mybir

    BF = mybir.dt.bfloat16
    F32 = mybir.dt.float32
    Exp = mybir.ActivationFunctionType.Exp
    N, C, H = _N, _C, _H
    BL = _BL

    nc = bass.Bass("TRN2", target_bir_lowering=False, debug=False)
    x_d = nc.dram_tensor("xt", [BL * C, N], BF, kind="ExternalInput").ap()
    wq_d = nc.dram_tensor("wq", [C, 3 * C], BF, kind="ExternalInput").ap()
    wp_d = nc.dram_tensor("wp", [C, C], BF, kind="ExternalInput").ap()
    cos_d = nc.dram_tensor("cosm", [BL * 128, N], BF, kind="ExternalInput").ap()
    sin_d = nc.dram_tensor("ssinm", [BL * 128, N], BF, kind="ExternalInput").ap()
    if has_bias:
        bqk_d = nc.dram_tensor("bqk", [1, 2 * C], BF, kind="ExternalInput").ap()
        bv_d = nc.dram_tensor("bv", [1, C], BF, kind="ExternalInput").ap()
        bp_d = nc.dram_tensor("bp", [1, C], BF, kind="ExternalInput").ap()
    out_d = nc.dram_tensor("out", [BL * N, C], BF, kind="ExternalOutput").ap()

    SH_MASK = [(i + 16) % 32 for i in range(32)]

    with tile.TileContext(nc) as tc, ExitStack() as ctx:
        const = ctx.enter_context(tc.tile_pool(name="const", bufs=1))
        xT_p = ctx.enter_context(tc.tile_pool(name="xT", bufs=12))
        map_p = ctx.enter_context(tc.tile_pool(name="maps", bufs=2))
        raw_p = ctx.enter_context(tc.tile_pool(name="qkraw", bufs=2))
        tmp_p = ctx.enter_context(tc.tile_pool(name="tmp", bufs=3))
        qkr_p = ctx.enter_context(tc.tile_pool(name="qkr", bufs=24))
        v_p = ctx.enter_context(tc.tile_pool(name="v", bufs=16))
        pT_p = ctx.enter_context(tc.tile_pool(name="pT", bufs=3))
        ao_p = ctx.enter_context(tc.tile_pool(name="ao", bufs=12))
        sum_p = ctx.enter_context(tc.tile_pool(name="sums", bufs=2))
        ost_p = ctx.enter_context(tc.tile_pool(name="ost", bufs=2))
        sc_ps = ctx.enter_context(tc.tile_pool(name="scps", bufs=2, space="PSUM"))
        bc_ps = ctx.enter_context(tc.tile_pool(name="bcps", bufs=1, space="PSUM"))
        o_ps = ctx.enter_context(tc.tile_pool(name="ops", bufs=2, space="PSUM"))

        # ---- resident constants ----
        ones64 = const.tile([1, 64], BF, tag="ones64", name="ones64")
        nc.gpsimd.memset(ones64[:], 1.0)
        wq_t = []
        for c in range(_NC6):
            t = const.tile([128, 3 * C], BF, tag=f"wq{c}", name=f"wq{c}")
            nc.sync.dma_start(
                t[:, 0:2 * C], wq_d[c * 128:(c + 1) * 128, 0:2 * C]
            )
            wq_t.append(t)
        if has_bias:
            bqk_sb = const.tile([1, 2 * C], BF, tag="bqk")
            nc.sync.dma_start(bqk_sb[:], bqk_d[:])
            bv_sb = const.tile([1, C], BF, tag="bv")
            nc.sync.dma_start(bv_sb[:], bv_d[:])
            bp_sb = const.tile([1, C], BF, tag="bp")
            nc.sync.dma_start(bp_sb[:], bp_d[:])
            ones_r = const.tile([1, 512], BF, tag="ones")
            nc.gpsimd.memset(ones_r[:], 1.0)

        # ---- inputs for both items up front (DMA overlaps compute) ----
        xT = []
        cosm = []
        ssin = []
        for b in range(BL):
            xTb = []
            for c in range(_NC6):
                t = xT_p.tile([128, N], BF, tag="xT", name=f"xT{b}_{c}")
                nc.sync.dma_start(
                    t[:], x_d[b * C + c * 128:b * C + (c + 1) * 128, :]
                )
                xTb.append(t)
            xT.append(xTb)
            cm = map_p.tile([128, N], BF, tag="cos", name=f"cos{b}")
            nc.sync.dma_start(cm[:], cos_d[b * 128:(b + 1) * 128, :])
            cosm.append(cm)
            sm = map_p.tile([128, N], BF, tag="sin", name=f"sin{b}")
            nc.sync.dma_start(sm[:], sin_d[b * 128:(b + 1) * 128, :])
            ssin.append(sm)

        # v-columns of wq and wp arrive after the hot qk inputs
        for c in range(_NC6):
            nc.sync.dma_start(
                wq_t[c][:, 2 * C:3 * C], wq_d[c * 128:(c + 1) * 128, 2 * C:3 * C]
            )
        wp_t = []
        for c in range(_NC6):
            t = const.tile([128, C], BF, tag=f"wp{c}", name=f"wp{c}")
            nc.sync.dma_start(t[:], wp_d[c * 128:(c + 1) * 128, :])
            wp_t.append(t)

        def emit_qk_feature(b, f):
            """q/k feature chunk: one stationary load per contraction chunk
            covers both 512-col halves of a 2-bank PSUM tile; fused RoPE."""
            raw = raw_p.tile([128, N], BF, tag="qkraw", name=f"raw{b}_{f}")
            ps = sc_ps.tile([128, N], F32, tag="sc", name=f"mq{b}_{f}")
            for c in range(_NC6):
                for t2 in range(2):
                    nc.tensor.matmul(
                        ps[:, t2 * 512:(t2 + 1) * 512],
                        wq_t[c][:, f * 128:(f + 1) * 128],
                        xT[b][c][:, t2 * 512:(t2 + 1) * 512],
                        start=(c == 0),
                        stop=(c == _NC6 - 1 and not has_bias),
                    )
            if has_bias:
                for t2 in range(2):
                    nc.tensor.matmul(
                        ps[:, t2 * 512:(t2 + 1) * 512],
                        bqk_sb[:, f * 128:(f + 1) * 128],
                        ones_r[:],
                        start=False,
                        stop=True,
                    )
            nc.scalar.copy(raw[:], ps[:])
            r = tmp_p.tile([128, N], BF, tag="ttmp", name=f"r{b}_{f}")
            nc.vector.stream_shuffle(r[:], raw[:], SH_MASK)
            tm = tmp_p.tile([128, N], BF, tag="ttmp", name=f"tm{b}_{f}")
            nc.vector.tensor_mul(tm[:], r[:], ssin[b][:])
            am = tmp_p.tile([128, N], BF, tag="ttmp", name=f"am{b}_{f}")
            nc.gpsimd.tensor_mul(am[:], raw[:], cosm[b][:])
            ro = qkr_p.tile([128, N], BF, tag="qkr", name=f"qkr{b}_{f}")
            nc.vector.tensor_add(ro[:], tm[:], am[:])
            return ro

        def emit_v(b, t8):
            """v token-major, per-head [v_h | ones] layout; one stationary
            (x chunk) load per contraction chunk covers all 768 v columns."""
            vt = v_p.tile([128, H * 65], BF, tag="v", name=f"v{b}_{t8}")
            vt3 = vt.rearrange("p (h w) -> p h w", w=65)
            nc.gpsimd.memset(vt3[:, :, 64:65], 1.0)
            ps = sc_ps.tile([128, N], F32, tag="sc", name=f"mv{b}_{t8}")
            for c in range(_NC6):
                for f0, fw in ((0, 512), (512, 256)):
                    nc.tensor.matmul(
                        ps[:, f0:f0 + fw],
                        xT[b][c][:, t8 * 128:(t8 + 1) * 128],
                        wq_t[c][:, 2 * C + f0:2 * C + f0 + fw],
                        start=(c == 0),
                        stop=(c == _NC6 - 1 and not has_bias),
                    )
            if has_bias:
                for f0, fw in ((0, 512), (512, 256)):
                    nc.tensor.matmul(
                        ps[:, f0:f0 + fw],
                        ones_r[:, 0:128],
                        bv_sb[:, f0:f0 + fw],
                        start=False,
                        stop=True,
                    )
            nc.scalar.copy(
                vt3[:, :, 0:64],
                ps[:, 0:768].rearrange("p (h w) -> p h w", w=64),
            )
            return vt

        def emit_head(b, h, qk_r, v_sb, ao_t, sums_st):
            """One attention head: scores -> exp -> attnv, k-chunk streamed.
            Denominator row (PSUM row 64) and ao rows drain on DVE."""
            jj, half = h // 2, (h % 2) * 64
            qh = qk_r[jj]
            kh = qk_r[6 + jj]
            op = [
                o_ps.tile([65, 512], F32, tag="o", name=f"op{b}_{h}_{i}")
                for i in range(2)
            ]
            pT_tiles = [None] * 8
            for kc in range(9):
                if kc < 8:
                    pT = pT_p.tile([128, N], BF, tag="pT", name=f"p{b}_{h}_{kc}")
                    s = sc_ps.tile([128, N], F32, tag="sc", name=f"s{b}_{h}_{kc}")
                    for qc in range(2):
                        nc.tensor.matmul(
                            s[:, qc * 512:(qc + 1) * 512],
                            kh[half:half + 64, kc * 128:(kc + 1) * 128],
                            qh[half:half + 64, qc * 512:(qc + 1) * 512],
                            start=True,
                            stop=True,
                        )
                    nc.scalar.activation(pT[:], s[:], Exp, scale=0.125)
                    pT_tiles[kc] = pT
                if kc >= 1:
                    kd = kc - 1
                    for qc in range(2):
                        nc.tensor.matmul(
                            op[qc][:],
                            v_sb[kd][:, h * 65:(h + 1) * 65],
                            pT_tiles[kd][:, qc * 512:(qc + 1) * 512],
                            start=(kd == 0),
                            stop=(kd == 7),
                        )
            srow = sum_p.tile([1, N], BF, tag="srow", bufs=1, name=f"sr{b}_{h}")
            for qc in range(2):
                nc.vector.tensor_copy(
                    srow[:, qc * 512:(qc + 1) * 512], op[qc][64:65, :]
                )
                nc.vector.tensor_copy(
                    ao_t[jj][half:half + 64, qc * 512:(qc + 1) * 512],
                    op[qc][0:64, :],
                )
            nc.gpsimd.dma_start(
                sums_st[8 * h:8 * (h + 1), :],
                srow[:].rearrange("p (a n) -> p a n", a=8),
            )

        def emit_recip(b, sums_st):
            """1/denominators for all 12 heads; each row is DMA'd back to a
            partition-0 tile so the broadcast matmul can use it as a moving
            operand (engine/matmul partition bases must be 0/32/64/96)."""
            sumf = sum_p.tile(
                [8 * H, N // 8], F32, tag="sumf", bufs=1, name=f"sf{b}"
            )
            nc.vector.tensor_copy(sumf[:], sums_st[:])
            recipf = sum_p.tile(
                [8 * H, N // 8], F32, tag="recipf", bufs=1, name=f"rf{b}"
            )
            nc.vector.reciprocal(recipf[:], sumf[:])
            recipb = sum_p.tile(
                [8 * H, N // 8], BF, tag="recipb", bufs=1, name=f"rb{b}"
            )
            nc.vector.tensor_copy(recipb[:], recipf[:])
            rows = []
            for h in range(H):
                rr = sum_p.tile(
                    [1, N], BF, tag="rbrow", bufs=4, name=f"rr{b}_{h}"
                )
                nc.gpsimd.dma_start(
                    rr[:].rearrange("p (a n) -> p a n", a=8),
                    recipb[8 * h:8 * (h + 1), :].rearrange(
                        "p (u n) -> p u n", u=1
                    ),
                )
                rows.append(rr)
            return rows

        def emit_norm(b, jj, recipb, ao_t):
            """Broadcast 1/denom rows for heads 2jj,2jj+1 across 64
            partitions each via ones-outer-product matmuls, then normalize
            the ao tile in place."""
            ps = bc_ps.tile([128, N], F32, tag="bcp", name=f"bcp{b}_{jj}")
            for k in range(2):
                for qc in range(2):
                    nc.tensor.matmul(
                        ps[k * 64:(k + 1) * 64, qc * 512:(qc + 1) * 512],
                        ones64[:],
                        recipb[2 * jj + k][:, qc * 512:(qc + 1) * 512],
                        start=True,
                        stop=True,
                    )
            nc.vector.tensor_mul(ao_t[jj][:], ao_t[jj][:], ps[:])

        def emit_proj(b, t8, ao_t, on_vec):
            """out[t8] = ao @ W_proj; one stationary (ao chunk) load per
            contraction chunk covers both 384-col output halves."""
            copy = nc.vector.tensor_copy if on_vec else nc.scalar.copy
            ot = ost_p.tile([128, C], BF, tag="ost", name=f"ost{b}_{t8}")
            ps = sc_ps.tile([128, N], F32, tag="sc", name=f"mp{b}_{t8}")
            for c in range(_NC6):
                for nf in range(2):
                    nc.tensor.matmul(
                        ps[:, nf * 512:nf * 512 + 384],
                        ao_t[c][:, t8 * 128:(t8 + 1) * 128],
                        wp_t[c][:, nf * 384:(nf + 1) * 384],
                        start=(c == 0),
                        stop=(c == _NC6 - 1 and not has_bias),
                    )
            if has_bias:
                for nf in range(2):
                    nc.tensor.matmul(
                        ps[:, nf * 512:nf * 512 + 384],
                        ones_r[:, 0:128],
                        bp_sb[:, nf * 384:(nf + 1) * 384],
                        start=False,
                        stop=True,
                    )
            for nf in range(2):
                copy(ot[:, nf * 384:(nf + 1) * 384],
                     ps[:, nf * 512:nf * 512 + 384])
            nc.sync.dma_start(
                out_d[b * N + t8 * 128:b * N + (t8 + 1) * 128, :], ot[:]
            )

        # ---- Phase A: qkv for both items (copies on idle ACT) ----
        qk_r = [[emit_qk_feature(b, f) for f in range(_NF12)]
                for b in range(BL)]
        v_sb = [[emit_v(b, t8) for t8 in range(8)] for b in range(BL)]

        # ---- Phase B: attention item0 ----
        sums0 = sum_p.tile([8 * H, N // 8], BF, tag="sums", name="sums0")
        ao0 = [
            ao_p.tile([128, N], BF, tag="ao", name=f"ao0_{i}")
            for i in range(_NC6)
        ]
        for h in range(H):
            emit_head(0, h, qk_r[0], v_sb[0], ao0, sums0)
        recipb0 = emit_recip(0, sums0)

        # ---- Phase C: attention item1, item0 normalize interleaved ----
        sums1 = sum_p.tile([8 * H, N // 8], BF, tag="sums", name="sums1")
        ao1 = [
            ao_p.tile([128, N], BF, tag="ao", name=f"ao1_{i}")
            for i in range(_NC6)
        ]
        for h in range(H):
            emit_head(1, h, qk_r[1], v_sb[1], ao1, sums1)
            if 1 <= h <= 6:
                emit_norm(0, h - 1, recipb0, ao0)
        recipb1 = emit_recip(1, sums1)

        # ---- Phase D: proj item0, item1 normalize, proj item1 ----
        for t8 in range(4):
            emit_proj(0, t8, ao0, on_vec=False)
        for jj in range(_NC6):
            emit_norm(1, jj, recipb1, ao1)
        for t8 in range(4, 8):
            emit_proj(0, t8, ao0, on_vec=False)
        for t8 in range(8):
            emit_proj(1, t8, ao1, on_vec=False)
    return _split_excess_waits(nc)


def _get_nc(has_bias):
    if has_bias not in _nc_cache:
        _nc_cache[has_bias] = _build(has_bias)
    return _nc_cache[has_bias]


def _prep_in_maps(x, W_qkv, b_qkv, W_proj, b_proj, pos_h, pos_w):
    import ml_dtypes

    bf16 = ml_dtypes.bfloat16
    has_bias = bool(np.any(b_qkv)) or bool(np.any(b_proj))

    inv = 1.0 / _ROPE_BASE ** (
        np.arange(0, _DR, 2, dtype=np.float32) / float(_DR)
    )  # [16]

    def rope_maps(pos):
        ang = pos.astype(np.float32)[..., None] * inv  # [B, N, 16]
        cos = np.repeat(np.cos(ang), 2, axis=-1)  # [B, N, 32]
        sin = np.repeat(np.sin(ang), 2, axis=-1)
        return cos.transpose(0, 2, 1), sin.transpose(0, 2, 1)  # [B, 32, N]

    ch, sh = rope_maps(np.asarray(pos_h))
    cw, sw = rope_maps(np.asarray(pos_w))
    cos64 = np.concatenate([ch, cw], axis=1)  # [B, 64, N]
    sin64 = np.concatenate([sh, sw], axis=1)
    sign = np.where((np.arange(64) % 32) < 16, -1.0, 1.0).astype(np.float32)
    ssin64 = sin64 * sign[None, :, None]
    cosm = np.tile(cos64, (1, 2, 1)).astype(bf16)  # [B, 128, N]
    ssinm = np.tile(ssin64, (1, 2, 1)).astype(bf16)

    xb = np.asarray(x).astype(bf16)
    xtb = xb.transpose(0, 2, 1)  # [B, C, N] feature-major per item
    wqb = np.ascontiguousarray(np.asarray(W_qkv).astype(bf16))
    wpb = np.ascontiguousarray(np.asarray(W_proj).astype(bf16))

    in_maps = []
    for i in range(_NCORES):
        lo, hi = i * _BL, (i + 1) * _BL
        m = {
            "xt": np.ascontiguousarray(xtb[lo:hi].reshape(_BL * _C, _N)),
            "wq": wqb,
            "wp": wpb,
            "cosm": np.ascontiguousarray(cosm[lo:hi].reshape(_BL * 128, _N)),
            "ssinm": np.ascontiguousarray(ssinm[lo:hi].reshape(_BL * 128, _N)),
        }
        if has_bias:
            bq = np.asarray(b_qkv).astype(bf16)
            m["bqk"] = np.ascontiguousarray(bq[:2 * _C].reshape(1, 2 * _C))
            m["bv"] = np.ascontiguousarray(bq[2 * _C:].reshape(1, _C))
            m["bp"] = np.ascontiguousarray(
                np.asarray(b_proj).astype(bf16).reshape(1, _C)
            )
        in_maps.append(m)
    return in_maps, has_bias


def _ensure_ntff_hook():
    """This image's antenv lacks axon_hooks; recreate it from the boot
    helper so run_bass_kernel_spmd(trace=True) can capture NTFF profiles."""
    import sys
    import types

    if "antenv.axon_hooks" in sys.modules:
        return
    try:
        from trn_agent_boot.trn_boot import _ntff_profile_via_ctypes

        hook = _ntff_profile_via_ctypes("/opt/axon/libaxon_pjrt.so")
    except Exception:
        hook = None
    mod = types.ModuleType("antenv.axon_hooks")
    mod._hook = hook
    mod.get_axon_ntff_profile_hook = lambda: mod._hook
    mod.set_axon_ntff_profile_hook = lambda h: setattr(mod, "_hook", h)
    sys.modules["antenv.axon_hooks"] = mod


def run(x, W_qkv, b_qkv, W_proj, b_proj, pos_h, pos_w, num_heads, **run_kwargs):
    """Build + execute on 8 NeuronCores; returns (output, BassKernelResults)."""
    from concourse.bass_utils import run_bass_kernel_spmd

    if run_kwargs.get("trace"):
        _ensure_ntff_hook()

    assert int(num_heads) == _H
    in_maps, has_bias = _prep_in_maps(
        x, W_qkv, b_qkv, W_proj, b_proj, pos_h, pos_w
    )
    nc = _get_nc(has_bias)
    res = run_bass_kernel_spmd(
        nc, in_maps, core_ids=list(range(_NCORES)), **run_kwargs
    )
    out = np.concatenate(
        [res.results[i]["out"].reshape(_BL, _N, _C) for i in range(_NCORES)],
        axis=0,
    ).astype(np.float32)
    return out, res


def kernel(x, W_qkv, b_qkv, W_proj, b_proj, pos_h, pos_w, num_heads):
    out, _ = run(x, W_qkv, b_qkv, W_proj, b_proj, pos_h, pos_w, num_heads)
    return out


# revision 17
# speedup vs baseline: 1.0449x; 1.0165x over previous
"""Trainium2 Bass kernel for nn_Attention_24704651887034.

Dense ViT-style attention block (B=16, N=1024, C=768, H=12 heads, 2D RoPE),
data-parallel over batch across 8 NeuronCores (2 batch items per core, no
collectives).

Per-core device kernel (all matmuls bf16, fp32 PSUM accumulation). All PSUM
accumulation groups run through [128,1024] 2-bank tiles so one stationary
load covers 1024 moving columns (walrus runs with ldw-opt disabled, so every
stationary switch costs ~110ns serially). Softmax normalization broadcasts
1/denom rows across partitions with a ones-outer-product matmul into PSUM
(a DMA row-broadcast costs ~11us; the matmul costs ~0.4us).

  Phase A: qk feature-major matmul + fused 2D RoPE and v token-major (with
           per-head [v_h | ones] column so attnv PSUM row 64 accumulates
           softmax denominators) for BOTH items; PSUM->SBUF copies on the
           otherwise-idle Scalar/ACT engine.
  Phase B: attention item0 (per head: sT=k'q'^T, Exp on ACT, outT=[v|1]pT),
           PSUM drains on DVE; per-head denominator row gathered to sums0.
  Phase C: attention item1, interleaved with item0's normalize (reciprocal
           on DVE + matmul row-broadcast + in-place ao multiply).
  Phase D: proj item0, item1 normalize, proj item1; proj copies on ACT.
"""

import numpy as np

_B, _N, _C, _H = 16, 1024, 768, 12
_HD, _DR = 64, 32
_ROPE_BASE = 10000.0
_NCORES = 8
_BL = _B // _NCORES  # batch items per core

_NC6 = _C // 128      # 6 contraction chunks
_NF12 = 2 * _C // 128  # 12 feature chunks for q,k

_nc_cache = {}


def _split_excess_waits(nc, max_waits=1):
    """Walrus in this toolchain accepts at most one sync-wait command per
    instruction; Tile's tail drain (and occasionally the scheduler) emits
    more. Split the excess onto same-engine NOPs inserted just before."""
    from concourse import mybir

    for f in nc.m.functions:
        for blk in f.blocks:
            insts = blk.instructions
            i = 0
            while i < len(insts):
                ins = insts[i]
                si = getattr(ins, "sync_info", None)
                if si is not None and len(si.on_wait) > max_waits:
                    excess = si.on_wait[max_waits:]
                    ins.sync_info = mybir.SyncInfo(
                        on_wait=list(si.on_wait[:max_waits]),
                        on_update=list(si.on_update),
                    )
                    for j, w in enumerate(excess):
                        nop = mybir.InstNoOp(
                            name=f"{ins.name}-sw{j}", engine=ins.engine
                        )
                        nop.sync_info = mybir.SyncInfo(on_wait=[w], on_update=[])
                        insts.insert(i, nop)
                        i += 1
                i += 1
    return nc


def _build(has_bias):
    from contextlib import ExitStack

    import concourse.bass as bass
    import concourse.tile as tile
    from concourse import mybir

    BF = mybir.dt.bfloat16
    F32 = mybir.dt.float32
    Exp = mybir.ActivationFunctionType.Exp
    N, C, H = _N, _C, _H
    BL = _BL

    nc = bass.Bass("TRN2", target_bir_lowering=False, debug=False)
    x_d = nc.dram_tensor("xt", [BL * C, N], BF, kind="ExternalInput").ap()
    wq_d = nc.dram_tensor("wq", [C, 3 * C], BF, kind="ExternalInput").ap()
    wp_d = nc.dram_tensor("wp", [C, C], BF, kind="ExternalInput").ap()
    cos_d = nc.dram_tensor("cosm", [BL * 128, N], BF, kind="ExternalInput").ap()
    sin_d = nc.dram_tensor("ssinm", [BL * 128, N], BF, kind="ExternalInput").ap()
    if has_bias:
        bqk_d = nc.dram_tensor("bqk", [1, 2 * C], BF, kind="ExternalInput").ap()
        bv_d = nc.dram_tensor("bv", [1, C], BF, kind="ExternalInput").ap()
        bp_d = nc.dram_tensor("bp", [1, C], BF, kind="ExternalInput").ap()
    out_d = nc.dram_tensor("out", [BL * N, C], BF, kind="ExternalOutput").ap()

    SH_MASK = [(i + 16) % 32 for i in range(32)]

    with tile.TileContext(nc) as tc, ExitStack() as ctx:
        const = ctx.enter_context(tc.tile_pool(name="const", bufs=1))
        xT_p = ctx.enter_context(tc.tile_pool(name="xT", bufs=12))
        map_p = ctx.enter_context(tc.tile_pool(name="maps", bufs=2))
        raw_p = ctx.enter_context(tc.tile_pool(name="qkraw", bufs=2))
        tmp_p = ctx.enter_context(tc.tile_pool(name="tmp", bufs=3))
        qkr_p = ctx.enter_context(tc.tile_pool(name="qkr", bufs=24))
        v_p = ctx.enter_context(tc.tile_pool(name="v", bufs=16))
        pT_p = ctx.enter_context(tc.tile_pool(name="pT", bufs=3))
        ao_p = ctx.enter_context(tc.tile_pool(name="ao", bufs=12))
        sum_p = ctx.enter_context(tc.tile_pool(name="sums", bufs=2))
        ost_p = ctx.enter_context(tc.tile_pool(name="ost", bufs=2))
        sc_ps = ctx.enter_context(tc.tile_pool(name="scps", bufs=2, space="PSUM"))
        bc_ps = ctx.enter_context(tc.tile_pool(name="bcps", bufs=1, space="PSUM"))
        o_ps = ctx.enter_context(tc.tile_pool(name="ops", bufs=2, space="PSUM"))

        # ---- resident constants ----
        ones64 = const.tile([1, 64], BF, tag="ones64", name="ones64")
        nc.gpsimd.memset(ones64[:], 1.0)
        wq_t = []
        for c in range(_NC6):
            t = const.tile([128, 3 * C], BF, tag=f"wq{c}", name=f"wq{c}")
            nc.sync.dma_start(
                t[:, 0:2 * C], wq_d[c * 128:(c + 1) * 128, 0:2 * C]
            )
            wq_t.append(t)
        if has_bias:
            bqk_sb = const.tile([1, 2 * C], BF, tag="bqk")
            nc.sync.dma_start(bqk_sb[:], bqk_d[:])
            bv_sb = const.tile([1, C], BF, tag="bv")
            nc.sync.dma_start(bv_sb[:], bv_d[:])
            bp_sb = const.tile([1, C], BF, tag="bp")
            nc.sync.dma_start(bp_sb[:], bp_d[:])
            ones_r = const.tile([1, 512], BF, tag="ones")
            nc.gpsimd.memset(ones_r[:], 1.0)

        # ---- inputs for both items up front (DMA overlaps compute) ----
        xT = []
        cosm = []
        ssin = []
        for b in range(BL):
            xTb = []
            for c in range(_NC6):
                t = xT_p.tile([128, N], BF, tag="xT", name=f"xT{b}_{c}")
                nc.sync.dma_start(
                    t[:], x_d[b * C + c * 128:b * C + (c + 1) * 128, :]
                )
                xTb.append(t)
            xT.append(xTb)
            cm = map_p.tile([128, N], BF, tag="cos", name=f"cos{b}")
            nc.sync.dma_start(cm[:], cos_d[b * 128:(b + 1) * 128, :])
            cosm.append(cm)
            sm = map_p.tile([128, N], BF, tag="sin", name=f"sin{b}")
            nc.sync.dma_start(sm[:], sin_d[b * 128:(b + 1) * 128, :])
            ssin.append(sm)

        # v-columns of wq and wp arrive after the hot qk inputs
        for c in range(_NC6):
            nc.sync.dma_start(
                wq_t[c][:, 2 * C:3 * C], wq_d[c * 128:(c + 1) * 128, 2 * C:3 * C]
            )
        wp_t = []
        for c in range(_NC6):
            t = const.tile([128, C], BF, tag=f"wp{c}", name=f"wp{c}")
            nc.sync.dma_start(t[:], wp_d[c * 128:(c + 1) * 128, :])
            wp_t.append(t)

        _grp = [0]

        def acc_tile(name):
            """Rotate qkv accumulation groups across 3 PSUM tiles (sc ring
            of 2 plus the broadcast bank) for 3-deep group pipelining."""
            _grp[0] += 1
            if _grp[0] % 3 == 0:
                return bc_ps.tile([128, N], F32, tag="bcp", name=name)
            return sc_ps.tile([128, N], F32, tag="sc", name=name)

        def emit_qk_feature(b, f):
            """q/k feature chunk: one stationary load per contraction chunk
            covers both 512-col halves of a 2-bank PSUM tile; fused RoPE."""
            raw = raw_p.tile([128, N], BF, tag="qkraw", name=f"raw{b}_{f}")
            ps = acc_tile(f"mq{b}_{f}")
            for c in range(_NC6):
                for t2 in range(2):
                    nc.tensor.matmul(
                        ps[:, t2 * 512:(t2 + 1) * 512],
                        wq_t[c][:, f * 128:(f + 1) * 128],
                        xT[b][c][:, t2 * 512:(t2 + 1) * 512],
                        start=(c == 0),
                        stop=(c == _NC6 - 1 and not has_bias),
                    )
            if has_bias:
                for t2 in range(2):
                    nc.tensor.matmul(
                        ps[:, t2 * 512:(t2 + 1) * 512],
                        bqk_sb[:, f * 128:(f + 1) * 128],
                        ones_r[:],
                        start=False,
                        stop=True,
                    )
            nc.scalar.copy(raw[:], ps[:])
            r = tmp_p.tile([128, N], BF, tag="ttmp", name=f"r{b}_{f}")
            nc.vector.stream_shuffle(r[:], raw[:], SH_MASK)
            tm = tmp_p.tile([128, N], BF, tag="ttmp", name=f"tm{b}_{f}")
            nc.vector.tensor_mul(tm[:], r[:], ssin[b][:])
            am = tmp_p.tile([128, N], BF, tag="ttmp", name=f"am{b}_{f}")
            nc.gpsimd.tensor_mul(am[:], raw[:], cosm[b][:])
            ro = qkr_p.tile([128, N], BF, tag="qkr", name=f"qkr{b}_{f}")
            nc.vector.tensor_add(ro[:], tm[:], am[:])
            return ro

        def emit_v(b, t8):
            """v token-major, per-head [v_h | ones] layout; one stationary
            (x chunk) load per contraction chunk covers all 768 v columns."""
            vt = v_p.tile([128, H * 65], BF, tag="v", name=f"v{b}_{t8}")
            vt3 = vt.rearrange("p (h w) -> p h w", w=65)
            nc.gpsimd.memset(vt3[:, :, 64:65], 1.0)
            ps = acc_tile(f"mv{b}_{t8}")
            for c in range(_NC6):
                for f0, fw in ((0, 512), (512, 256)):
                    nc.tensor.matmul(
                        ps[:, f0:f0 + fw],
                        xT[b][c][:, t8 * 128:(t8 + 1) * 128],
                        wq_t[c][:, 2 * C + f0:2 * C + f0 + fw],
                        start=(c == 0),
                        stop=(c == _NC6 - 1 and not has_bias),
                    )
            if has_bias:
                for f0, fw in ((0, 512), (512, 256)):
                    nc.tensor.matmul(
                        ps[:, f0:f0 + fw],
                        ones_r[:, 0:128],
                        bv_sb[:, f0:f0 + fw],
                        start=False,
                        stop=True,
                    )
            nc.scalar.copy(
                vt3[:, :, 0:64],
                ps[:, 0:768].rearrange("p (h w) -> p h w", w=64),
            )
            return vt

        def emit_head(b, h, qk_r, v_sb, ao_t, sums_st):
            """One attention head: scores -> exp -> attnv, k-chunk streamed.
            Denominator row (PSUM row 64) and ao rows drain on DVE."""
            jj, half = h // 2, (h % 2) * 64
            qh = qk_r[jj]
            kh = qk_r[6 + jj]
            op = [
                o_ps.tile([65, 512], F32, tag="o", name=f"op{b}_{h}_{i}")
                for i in range(2)
            ]
            pT_tiles = [None] * 8
            for kc in range(9):
                if kc < 8:
                    pT = pT_p.tile([128, N], BF, tag="pT", name=f"p{b}_{h}_{kc}")
                    s = sc_ps.tile([128, N], F32, tag="sc", name=f"s{b}_{h}_{kc}")
                    for qc in range(2):
                        nc.tensor.matmul(
                            s[:, qc * 512:(qc + 1) * 512],
                            kh[half:half + 64, kc * 128:(kc + 1) * 128],
                            qh[half:half + 64, qc * 512:(qc + 1) * 512],
                            start=True,
                            stop=True,
                        )
                    nc.scalar.activation(pT[:], s[:], Exp, scale=0.125)
                    pT_tiles[kc] = pT
                if kc >= 1:
                    kd = kc - 1
                    for qc in range(2):
                        nc.tensor.matmul(
                            op[qc][:],
                            v_sb[kd][:, h * 65:(h + 1) * 65],
                            pT_tiles[kd][:, qc * 512:(qc + 1) * 512],
                            start=(kd == 0),
                            stop=(kd == 7),
                        )
            srow = sum_p.tile([1, N], BF, tag="srow", bufs=1, name=f"sr{b}_{h}")
            for qc in range(2):
                nc.vector.tensor_copy(
                    srow[:, qc * 512:(qc + 1) * 512], op[qc][64:65, :]
                )
                nc.vector.tensor_copy(
                    ao_t[jj][half:half + 64, qc * 512:(qc + 1) * 512],
                    op[qc][0:64, :],
                )
            nc.gpsimd.dma_start(
                sums_st[8 * h:8 * (h + 1), :],
                srow[:].rearrange("p (a n) -> p a n", a=8),
            )

        def emit_recip(b, sums_st):
            """1/denominators for all 12 heads; each row is DMA'd back to a
            partition-0 tile so the broadcast matmul can use it as a moving
            operand (engine/matmul partition bases must be 0/32/64/96)."""
            sumf = sum_p.tile(
                [8 * H, N // 8], F32, tag="sumf", bufs=1, name=f"sf{b}"
            )
            nc.vector.tensor_copy(sumf[:], sums_st[:])
            recipf = sum_p.tile(
                [8 * H, N // 8], F32, tag="recipf", bufs=1, name=f"rf{b}"
            )
            nc.vector.reciprocal(recipf[:], sumf[:])
            recipb = sum_p.tile(
                [8 * H, N // 8], BF, tag="recipb", bufs=1, name=f"rb{b}"
            )
            nc.vector.tensor_copy(recipb[:], recipf[:])
            rows = []
            for h in range(H):
                rr = sum_p.tile(
                    [1, N], BF, tag="rbrow", bufs=4, name=f"rr{b}_{h}"
                )
                nc.gpsimd.dma_start(
                    rr[:].rearrange("p (a n) -> p a n", a=8),
                    recipb[8 * h:8 * (h + 1), :].rearrange(
                        "p (u n) -> p u n", u=1
                    ),
                )
                rows.append(rr)
            return rows

        def emit_norm(b, jj, recipb, ao_t):
            """Broadcast 1/denom rows for heads 2jj,2jj+1 across 64
            partitions each via ones-outer-product matmuls, then normalize
            the ao tile in place."""
            ps = bc_ps.tile([128, N], F32, tag="bcp", name=f"bcp{b}_{jj}")
            for k in range(2):
                for qc in range(2):
                    nc.tensor.matmul(
                        ps[k * 64:(k + 1) * 64, qc * 512:(qc + 1) * 512],
                        ones64[:],
                        recipb[2 * jj + k][:, qc * 512:(qc + 1) * 512],
                        start=True,
                        stop=True,
                    )
            nc.vector.tensor_mul(ao_t[jj][:], ao_t[jj][:], ps[:])

        def emit_proj(b, t8, ao_t, on_vec):
            """out[t8] = ao @ W_proj; one stationary (ao chunk) load per
            contraction chunk covers both 384-col output halves."""
            copy = nc.vector.tensor_copy if on_vec else nc.scalar.copy
            ot = ost_p.tile([128, C], BF, tag="ost", name=f"ost{b}_{t8}")
            ps = sc_ps.tile([128, N], F32, tag="sc", name=f"mp{b}_{t8}")
            for c in range(_NC6):
                for nf in range(2):
                    nc.tensor.matmul(
                        ps[:, nf * 512:nf * 512 + 384],
                        ao_t[c][:, t8 * 128:(t8 + 1) * 128],
                        wp_t[c][:, nf * 384:(nf + 1) * 384],
                        start=(c == 0),
                        stop=(c == _NC6 - 1 and not has_bias),
                    )
            if has_bias:
                for nf in range(2):
                    nc.tensor.matmul(
                        ps[:, nf * 512:nf * 512 + 384],
                        ones_r[:, 0:128],
                        bp_sb[:, nf * 384:(nf + 1) * 384],
                        start=False,
                        stop=True,
                    )
            for nf in range(2):
                copy(ot[:, nf * 384:(nf + 1) * 384],
                     ps[:, nf * 512:nf * 512 + 384])
            nc.sync.dma_start(
                out_d[b * N + t8 * 128:b * N + (t8 + 1) * 128, :], ot[:]
            )

        # ---- Phase A: qkv for both items (copies on idle ACT) ----
        qk_r = [[emit_qk_feature(b, f) for f in range(_NF12)]
                for b in range(BL)]
        v_sb = [[emit_v(b, t8) for t8 in range(8)] for b in range(BL)]

        # ---- Phase B: attention item0 ----
        sums0 = sum_p.tile([8 * H, N // 8], BF, tag="sums", name="sums0")
        ao0 = [
            ao_p.tile([128, N], BF, tag="ao", name=f"ao0_{i}")
            for i in range(_NC6)
        ]
        for h in range(H):
            emit_head(0, h, qk_r[0], v_sb[0], ao0, sums0)
        recipb0 = emit_recip(0, sums0)

        # ---- Phase C: attention item1, item0 normalize interleaved ----
        sums1 = sum_p.tile([8 * H, N // 8], BF, tag="sums", name="sums1")
        ao1 = [
            ao_p.tile([128, N], BF, tag="ao", name=f"ao1_{i}")
            for i in range(_NC6)
        ]
        for h in range(H):
            emit_head(1, h, qk_r[1], v_sb[1], ao1, sums1)
            if 1 <= h <= 6:
                emit_norm(0, h - 1, recipb0, ao0)
        recipb1 = emit_recip(1, sums1)

        # ---- Phase D: proj item0 (dense, on sc ring) with item1's
        # normalize interleaved on the bc bank, then proj item1 ----
        for t8 in range(8):
            emit_proj(0, t8, ao0, on_vec=False)
            if 2 <= t8 < 8 and t8 - 2 < _NC6:
                emit_norm(1, t8 - 2, recipb1, ao1)
        for t8 in range(8):
            emit_proj(1, t8, ao1, on_vec=(t8 % 2 == 1))
    return _split_excess_waits(nc)


def _get_nc(has_bias):
    if has_bias not in _nc_cache:
        _nc_cache[has_bias] = _build(has_bias)
    return _nc_cache[has_bias]


def _prep_in_maps(x, W_qkv, b_qkv, W_proj, b_proj, pos_h, pos_w):
    import ml_dtypes

    bf16 = ml_dtypes.bfloat16
    has_bias = bool(np.any(b_qkv)) or bool(np.any(b_proj))

    inv = 1.0 / _ROPE_BASE ** (
        np.arange(0, _DR, 2, dtype=np.float32) / float(_DR)
    )  # [16]

    def rope_maps(pos):
        ang = pos.astype(np.float32)[..., None] * inv  # [B, N, 16]
        cos = np.repeat(np.cos(ang), 2, axis=-1)  # [B, N, 32]
        sin = np.repeat(np.sin(ang), 2, axis=-1)
        return cos.transpose(0, 2, 1), sin.transpose(0, 2, 1)  # [B, 32, N]

    ch, sh = rope_maps(np.asarray(pos_h))
    cw, sw = rope_maps(np.asarray(pos_w))
    cos64 = np.concatenate([ch, cw], axis=1)  # [B, 64, N]
    sin64 = np.concatenate([sh, sw], axis=1)
    sign = np.where((np.arange(64) % 32) < 16, -1.0, 1.0).astype(np.float32)
    ssin64 = sin64 * sign[None, :, None]
    cosm = np.tile(cos64, (1, 2, 1)).astype(bf16)  # [B, 128, N]
    ssinm = np.tile(ssin64, (1, 2, 1)).astype(bf16)

    xb = np.asarray(x).astype(bf16)
    xtb = xb.transpose(0, 2, 1)  # [B, C, N] feature-major per item
    wqb = np.ascontiguousarray(np.asarray(W_qkv).astype(bf16))
    wpb = np.ascontiguousarray(np.asarray(W_proj).astype(bf16))

    in_maps = []
    for i in range(_NCORES):
        lo, hi = i * _BL, (i + 1) * _BL
        m = {
            "xt": np.ascontiguousarray(xtb[lo:hi].reshape(_BL * _C, _N)),
            "wq": wqb,
            "wp": wpb,
            "cosm": np.ascontiguousarray(cosm[lo:hi].reshape(_BL * 128, _N)),
            "ssinm": np.ascontiguousarray(ssinm[lo:hi].reshape(_BL * 128, _N)),
        }
        if has_bias:
            bq = np.asarray(b_qkv).astype(bf16)
            m["bqk"] = np.ascontiguousarray(bq[:2 * _C].reshape(1, 2 * _C))
            m["bv"] = np.ascontiguousarray(bq[2 * _C:].reshape(1, _C))
            m["bp"] = np.ascontiguousarray(
                np.asarray(b_proj).astype(bf16).reshape(1, _C)
            )
        in_maps.append(m)
    return in_maps, has_bias


def _ensure_ntff_hook():
    """This image's antenv lacks axon_hooks; recreate it from the boot
    helper so run_bass_kernel_spmd(trace=True) can capture NTFF profiles."""
    import sys
    import types

    if "antenv.axon_hooks" in sys.modules:
        return
    try:
        from trn_agent_boot.trn_boot import _ntff_profile_via_ctypes

        hook = _ntff_profile_via_ctypes("/opt/axon/libaxon_pjrt.so")
    except Exception:
        hook = None
    mod = types.ModuleType("antenv.axon_hooks")
    mod._hook = hook
    mod.get_axon_ntff_profile_hook = lambda: mod._hook
    mod.set_axon_ntff_profile_hook = lambda h: setattr(mod, "_hook", h)
    sys.modules["antenv.axon_hooks"] = mod


def run(x, W_qkv, b_qkv, W_proj, b_proj, pos_h, pos_w, num_heads, **run_kwargs):
    """Build + execute on 8 NeuronCores; returns (output, BassKernelResults)."""
    from concourse.bass_utils import run_bass_kernel_spmd

    if run_kwargs.get("trace"):
        _ensure_ntff_hook()

    assert int(num_heads) == _H
    in_maps, has_bias = _prep_in_maps(
        x, W_qkv, b_qkv, W_proj, b_proj, pos_h, pos_w
    )
    nc = _get_nc(has_bias)
    res = run_bass_kernel_spmd(
        nc, in_maps, core_ids=list(range(_NCORES)), **run_kwargs
    )
    out = np.concatenate(
        [res.results[i]["out"].reshape(_BL, _N, _C) for i in range(_NCORES)],
        axis=0,
    ).astype(np.float32)
    return out, res


def kernel(x, W_qkv, b_qkv, W_proj, b_proj, pos_h, pos_w, num_heads):
    out, _ = run(x, W_qkv, b_qkv, W_proj, b_proj, pos_h, pos_w, num_heads)
    return out


# revision 19
# speedup vs baseline: 1.1152x; 1.0673x over previous
"""Trainium2 Bass kernel for nn_Attention_24704651887034.

Dense ViT-style attention block (B=16, N=1024, C=768, H=12 heads, 2D RoPE),
data-parallel over batch across 8 NeuronCores (2 batch items per core, no
collectives).

Per-core device kernel (all matmuls bf16, fp32 PSUM accumulation). All PSUM
accumulation groups run through [128,1024] 2-bank tiles so one stationary
load covers 1024 moving columns (walrus runs with ldw-opt disabled, so every
stationary switch costs ~110ns serially). Softmax normalization broadcasts
1/denom rows across partitions with a ones-outer-product matmul into PSUM
(a DMA row-broadcast costs ~11us; the matmul costs ~0.4us).

  Phase A: qk feature-major matmul + fused 2D RoPE and v token-major (with
           per-head [v_h | ones] column so attnv PSUM row 64 accumulates
           softmax denominators) for BOTH items; PSUM->SBUF copies on the
           otherwise-idle Scalar/ACT engine.
  Phase B: attention item0 (per head: sT=k'q'^T, Exp on ACT, outT=[v|1]pT),
           PSUM drains on DVE; per-head denominator row gathered to sums0.
  Phase C: attention item1, interleaved with item0's normalize (reciprocal
           on DVE + matmul row-broadcast + in-place ao multiply).
  Phase D: proj item0, item1 normalize, proj item1; proj copies on ACT.
"""

import numpy as np

_B, _N, _C, _H = 16, 1024, 768, 12
_HD, _DR = 64, 32
_ROPE_BASE = 10000.0
_NCORES = 8
_BL = _B // _NCORES  # batch items per core

_NC6 = _C // 128      # 6 contraction chunks
_NF12 = 2 * _C // 128  # 12 feature chunks for q,k

_nc_cache = {}


def _split_excess_waits(nc, max_waits=1):
    """Walrus in this toolchain accepts at most one sync-wait command per
    instruction; Tile's tail drain (and occasionally the scheduler) emits
    more. Split the excess onto same-engine NOPs inserted just before."""
    from concourse import mybir

    for f in nc.m.functions:
        for blk in f.blocks:
            insts = blk.instructions
            i = 0
            while i < len(insts):
                ins = insts[i]
                si = getattr(ins, "sync_info", None)
                if si is not None and len(si.on_wait) > max_waits:
                    excess = si.on_wait[max_waits:]
                    ins.sync_info = mybir.SyncInfo(
                        on_wait=list(si.on_wait[:max_waits]),
                        on_update=list(si.on_update),
                    )
                    for j, w in enumerate(excess):
                        nop = mybir.InstNoOp(
                            name=f"{ins.name}-sw{j}", engine=ins.engine
                        )
                        nop.sync_info = mybir.SyncInfo(on_wait=[w], on_update=[])
                        insts.insert(i, nop)
                        i += 1
                i += 1
    return nc


def _build(has_bias):
    from contextlib import ExitStack

    import concourse.bass as bass
    import concourse.tile as tile
    from concourse import mybir

    BF = mybir.dt.bfloat16
    F32 = mybir.dt.float32
    Exp = mybir.ActivationFunctionType.Exp
    N, C, H = _N, _C, _H
    BL = _BL

    nc = bass.Bass("TRN2", target_bir_lowering=False, debug=False)
    x_d = nc.dram_tensor("xt", [BL * C, N], BF, kind="ExternalInput").ap()
    wq_d = nc.dram_tensor("wq", [C, 3 * C], BF, kind="ExternalInput").ap()
    wp_d = nc.dram_tensor("wp", [C, C], BF, kind="ExternalInput").ap()
    cos_d = nc.dram_tensor("cosm", [BL * 128, N], BF, kind="ExternalInput").ap()
    sin_d = nc.dram_tensor("ssinm", [BL * 128, N], BF, kind="ExternalInput").ap()
    if has_bias:
        bqk_d = nc.dram_tensor("bqk", [1, 2 * C], BF, kind="ExternalInput").ap()
        bv_d = nc.dram_tensor("bv", [1, C], BF, kind="ExternalInput").ap()
        bp_d = nc.dram_tensor("bp", [1, C], BF, kind="ExternalInput").ap()
    out_d = nc.dram_tensor("out", [BL * N, C], BF, kind="ExternalOutput").ap()

    SH_MASK = [(i + 16) % 32 for i in range(32)]

    with tile.TileContext(nc) as tc, ExitStack() as ctx:
        const = ctx.enter_context(tc.tile_pool(name="const", bufs=1))
        xT_p = ctx.enter_context(tc.tile_pool(name="xT", bufs=12))
        map_p = ctx.enter_context(tc.tile_pool(name="maps", bufs=2))
        raw_p = ctx.enter_context(tc.tile_pool(name="qkraw", bufs=2))
        tmp_p = ctx.enter_context(tc.tile_pool(name="tmp", bufs=3))
        qkr_p = ctx.enter_context(tc.tile_pool(name="qkr", bufs=24))
        v_p = ctx.enter_context(tc.tile_pool(name="v", bufs=16))
        pT_p = ctx.enter_context(tc.tile_pool(name="pT", bufs=3))
        ao_p = ctx.enter_context(tc.tile_pool(name="ao", bufs=12))
        sum_p = ctx.enter_context(tc.tile_pool(name="sums", bufs=2))
        ost_p = ctx.enter_context(tc.tile_pool(name="ost", bufs=2))
        sc_ps = ctx.enter_context(tc.tile_pool(name="scps", bufs=2, space="PSUM"))
        bc_ps = ctx.enter_context(tc.tile_pool(name="bcps", bufs=1, space="PSUM"))
        o_ps = ctx.enter_context(tc.tile_pool(name="ops", bufs=2, space="PSUM"))

        # ---- resident constants ----
        ones64 = const.tile([1, 64], BF, tag="ones64", name="ones64")
        nc.gpsimd.memset(ones64[:], 1.0)
        wq_t = []
        for c in range(_NC6):
            t = const.tile([128, 3 * C], BF, tag=f"wq{c}", name=f"wq{c}")
            nc.sync.dma_start(
                t[:, 0:2 * C], wq_d[c * 128:(c + 1) * 128, 0:2 * C]
            )
            wq_t.append(t)
        if has_bias:
            bqk_sb = const.tile([1, 2 * C], BF, tag="bqk")
            nc.sync.dma_start(bqk_sb[:], bqk_d[:])
            bv_sb = const.tile([1, C], BF, tag="bv")
            nc.sync.dma_start(bv_sb[:], bv_d[:])
            bp_sb = const.tile([1, C], BF, tag="bp")
            nc.sync.dma_start(bp_sb[:], bp_d[:])
            ones_r = const.tile([1, 512], BF, tag="ones")
            nc.gpsimd.memset(ones_r[:], 1.0)

        # ---- inputs for both items up front (DMA overlaps compute) ----
        xT = []
        cosm = []
        ssin = []
        for b in range(BL):
            xTb = []
            for c in range(_NC6):
                t = xT_p.tile([128, N], BF, tag="xT", name=f"xT{b}_{c}")
                nc.sync.dma_start(
                    t[:], x_d[b * C + c * 128:b * C + (c + 1) * 128, :]
                )
                xTb.append(t)
            xT.append(xTb)
            cm = map_p.tile([128, N], BF, tag="cos", name=f"cos{b}")
            nc.sync.dma_start(cm[:], cos_d[b * 128:(b + 1) * 128, :])
            cosm.append(cm)
            sm = map_p.tile([128, N], BF, tag="sin", name=f"sin{b}")
            nc.sync.dma_start(sm[:], sin_d[b * 128:(b + 1) * 128, :])
            ssin.append(sm)

        # v-columns of wq and wp arrive after the hot qk inputs
        for c in range(_NC6):
            nc.sync.dma_start(
                wq_t[c][:, 2 * C:3 * C], wq_d[c * 128:(c + 1) * 128, 2 * C:3 * C]
            )
        wp_t = []
        for c in range(_NC6):
            t = const.tile([128, C], BF, tag=f"wp{c}", name=f"wp{c}")
            nc.sync.dma_start(t[:], wp_d[c * 128:(c + 1) * 128, :])
            wp_t.append(t)

        _grp = [0]

        def acc_tile(name):
            """Rotate qkv accumulation groups across 3 PSUM tiles (sc ring
            of 2 plus the broadcast bank) for 3-deep group pipelining."""
            _grp[0] += 1
            if _grp[0] % 3 == 0:
                return bc_ps.tile([128, N], F32, tag="bcp", name=name)
            return sc_ps.tile([128, N], F32, tag="sc", name=name)

        def emit_qk_feature(b, f):
            """q/k feature chunk: one stationary load per contraction chunk
            covers both 512-col halves of a 2-bank PSUM tile; fused RoPE."""
            raw = raw_p.tile([128, N], BF, tag="qkraw", name=f"raw{b}_{f}")
            ps = acc_tile(f"mq{b}_{f}")
            for c in range(_NC6):
                for t2 in range(2):
                    nc.tensor.matmul(
                        ps[:, t2 * 512:(t2 + 1) * 512],
                        wq_t[c][:, f * 128:(f + 1) * 128],
                        xT[b][c][:, t2 * 512:(t2 + 1) * 512],
                        start=(c == 0),
                        stop=(c == _NC6 - 1 and not has_bias),
                    )
            if has_bias:
                for t2 in range(2):
                    nc.tensor.matmul(
                        ps[:, t2 * 512:(t2 + 1) * 512],
                        bqk_sb[:, f * 128:(f + 1) * 128],
                        ones_r[:],
                        start=False,
                        stop=True,
                    )
            nc.scalar.copy(raw[:], ps[:])
            r = tmp_p.tile([128, N], BF, tag="ttmp", name=f"r{b}_{f}")
            nc.vector.stream_shuffle(r[:], raw[:], SH_MASK)
            tm = tmp_p.tile([128, N], BF, tag="ttmp", name=f"tm{b}_{f}")
            nc.vector.tensor_mul(tm[:], r[:], ssin[b][:])
            am = tmp_p.tile([128, N], BF, tag="ttmp", name=f"am{b}_{f}")
            nc.gpsimd.tensor_mul(am[:], raw[:], cosm[b][:])
            ro = qkr_p.tile([128, N], BF, tag="qkr", name=f"qkr{b}_{f}")
            nc.vector.tensor_add(ro[:], tm[:], am[:])
            return ro

        def emit_v(b, t8):
            """v token-major, per-head [v_h | ones] layout; one stationary
            (x chunk) load per contraction chunk covers all 768 v columns."""
            vt = v_p.tile([128, H * 65], BF, tag="v", name=f"v{b}_{t8}")
            vt3 = vt.rearrange("p (h w) -> p h w", w=65)
            nc.gpsimd.memset(vt3[:, :, 64:65], 1.0)
            ps = acc_tile(f"mv{b}_{t8}")
            for c in range(_NC6):
                for f0, fw in ((0, 512), (512, 256)):
                    nc.tensor.matmul(
                        ps[:, f0:f0 + fw],
                        xT[b][c][:, t8 * 128:(t8 + 1) * 128],
                        wq_t[c][:, 2 * C + f0:2 * C + f0 + fw],
                        start=(c == 0),
                        stop=(c == _NC6 - 1 and not has_bias),
                    )
            if has_bias:
                for f0, fw in ((0, 512), (512, 256)):
                    nc.tensor.matmul(
                        ps[:, f0:f0 + fw],
                        ones_r[:, 0:128],
                        bv_sb[:, f0:f0 + fw],
                        start=False,
                        stop=True,
                    )
            nc.scalar.copy(
                vt3[:, :, 0:64],
                ps[:, 0:768].rearrange("p (h w) -> p h w", w=64),
            )
            return vt

        def emit_head(b, h, qk_r, v_sb, ao_t, sums_st):
            """One attention head: scores -> exp -> attnv, k-chunk streamed.
            Denominator row (PSUM row 64) and ao rows drain on DVE."""
            jj, half = h // 2, (h % 2) * 64
            qh = qk_r[jj]
            kh = qk_r[6 + jj]
            op = [
                o_ps.tile([65, 512], F32, tag="o", name=f"op{b}_{h}_{i}")
                for i in range(2)
            ]
            pT_tiles = [None] * 8
            for kc in range(9):
                if kc < 8:
                    pT = pT_p.tile([128, N], BF, tag="pT", name=f"p{b}_{h}_{kc}")
                    if b == 0:
                        s = acc_tile(f"s{b}_{h}_{kc}")
                    else:
                        s = sc_ps.tile(
                            [128, N], F32, tag="sc", name=f"s{b}_{h}_{kc}"
                        )
                    for qc in range(2):
                        nc.tensor.matmul(
                            s[:, qc * 512:(qc + 1) * 512],
                            kh[half:half + 64, kc * 128:(kc + 1) * 128],
                            qh[half:half + 64, qc * 512:(qc + 1) * 512],
                            start=True,
                            stop=True,
                        )
                    nc.scalar.activation(pT[:], s[:], Exp, scale=0.125)
                    pT_tiles[kc] = pT
                if kc >= 1:
                    kd = kc - 1
                    for qc in range(2):
                        nc.tensor.matmul(
                            op[qc][:],
                            v_sb[kd][:, h * 65:(h + 1) * 65],
                            pT_tiles[kd][:, qc * 512:(qc + 1) * 512],
                            start=(kd == 0),
                            stop=(kd == 7),
                        )
            srow = sum_p.tile([1, N], BF, tag="srow", bufs=1, name=f"sr{b}_{h}")
            for qc in range(2):
                nc.vector.tensor_copy(
                    srow[:, qc * 512:(qc + 1) * 512], op[qc][64:65, :]
                )
                nc.vector.tensor_copy(
                    ao_t[jj][half:half + 64, qc * 512:(qc + 1) * 512],
                    op[qc][0:64, :],
                )
            nc.gpsimd.dma_start(
                sums_st[8 * h:8 * (h + 1), :],
                srow[:].rearrange("p (a n) -> p a n", a=8),
            )

        def emit_recip(b, sums_st):
            """1/denominators for all 12 heads; each row is DMA'd back to a
            partition-0 tile so the broadcast matmul can use it as a moving
            operand (engine/matmul partition bases must be 0/32/64/96)."""
            sumf = sum_p.tile(
                [8 * H, N // 8], F32, tag="sumf", bufs=1, name=f"sf{b}"
            )
            nc.vector.tensor_copy(sumf[:], sums_st[:])
            recipf = sum_p.tile(
                [8 * H, N // 8], F32, tag="recipf", bufs=1, name=f"rf{b}"
            )
            nc.vector.reciprocal(recipf[:], sumf[:])
            recipb = sum_p.tile(
                [8 * H, N // 8], BF, tag="recipb", bufs=1, name=f"rb{b}"
            )
            nc.vector.tensor_copy(recipb[:], recipf[:])
            rows = []
            for h in range(H):
                rr = sum_p.tile(
                    [1, N], BF, tag="rbrow", bufs=4, name=f"rr{b}_{h}"
                )
                nc.gpsimd.dma_start(
                    rr[:].rearrange("p (a n) -> p a n", a=8),
                    recipb[8 * h:8 * (h + 1), :].rearrange(
                        "p (u n) -> p u n", u=1
                    ),
                )
                rows.append(rr)
            return rows

        def emit_norm(b, jj, recipb, ao_t):
            """Broadcast 1/denom rows for heads 2jj,2jj+1 across 64
            partitions each via ones-outer-product matmuls, then normalize
            the ao tile in place."""
            ps = bc_ps.tile([128, N], F32, tag="bcp", name=f"bcp{b}_{jj}")
            for k in range(2):
                for qc in range(2):
                    nc.tensor.matmul(
                        ps[k * 64:(k + 1) * 64, qc * 512:(qc + 1) * 512],
                        ones64[:],
                        recipb[2 * jj + k][:, qc * 512:(qc + 1) * 512],
                        start=True,
                        stop=True,
                    )
            nc.vector.tensor_mul(ao_t[jj][:], ao_t[jj][:], ps[:])

        def emit_proj(b, t8, ao_t, on_vec):
            """out[t8] = ao @ W_proj; one stationary (ao chunk) load per
            contraction chunk covers both 384-col output halves."""
            copy = nc.vector.tensor_copy if on_vec else nc.scalar.copy
            ot = ost_p.tile([128, C], BF, tag="ost", name=f"ost{b}_{t8}")
            ps = sc_ps.tile([128, N], F32, tag="sc", name=f"mp{b}_{t8}")
            for c in range(_NC6):
                for nf in range(2):
                    nc.tensor.matmul(
                        ps[:, nf * 512:nf * 512 + 384],
                        ao_t[c][:, t8 * 128:(t8 + 1) * 128],
                        wp_t[c][:, nf * 384:(nf + 1) * 384],
                        start=(c == 0),
                        stop=(c == _NC6 - 1 and not has_bias),
                    )
            if has_bias:
                for nf in range(2):
                    nc.tensor.matmul(
                        ps[:, nf * 512:nf * 512 + 384],
                        ones_r[:, 0:128],
                        bp_sb[:, nf * 384:(nf + 1) * 384],
                        start=False,
                        stop=True,
                    )
            for nf in range(2):
                copy(ot[:, nf * 384:(nf + 1) * 384],
                     ps[:, nf * 512:nf * 512 + 384])
            nc.sync.dma_start(
                out_d[b * N + t8 * 128:b * N + (t8 + 1) * 128, :], ot[:]
            )

        # ---- Phase A: qkv for both items (copies on idle ACT) ----
        qk_r = [[emit_qk_feature(b, f) for f in range(_NF12)]
                for b in range(BL)]
        v_sb = [[emit_v(b, t8) for t8 in range(8)] for b in range(BL)]

        # ---- Phase B: attention item0 ----
        sums0 = sum_p.tile([8 * H, N // 8], BF, tag="sums", name="sums0")
        ao0 = [
            ao_p.tile([128, N], BF, tag="ao", name=f"ao0_{i}")
            for i in range(_NC6)
        ]
        for h in range(H):
            emit_head(0, h, qk_r[0], v_sb[0], ao0, sums0)
        recipb0 = emit_recip(0, sums0)

        # ---- Phase C: attention item1, item0 normalize interleaved ----
        sums1 = sum_p.tile([8 * H, N // 8], BF, tag="sums", name="sums1")
        ao1 = [
            ao_p.tile([128, N], BF, tag="ao", name=f"ao1_{i}")
            for i in range(_NC6)
        ]
        for h in range(H):
            emit_head(1, h, qk_r[1], v_sb[1], ao1, sums1)
            if 1 <= h <= 6:
                emit_norm(0, h - 1, recipb0, ao0)
        recipb1 = emit_recip(1, sums1)

        # ---- Phase D: proj item0 (dense, on sc ring) with item1's
        # normalize interleaved on the bc bank, then proj item1 ----
        for t8 in range(8):
            emit_proj(0, t8, ao0, on_vec=False)
            if 2 <= t8 < 8 and t8 - 2 < _NC6:
                emit_norm(1, t8 - 2, recipb1, ao1)
        for t8 in range(8):
            emit_proj(1, t8, ao1, on_vec=(t8 % 2 == 1))
    return _split_excess_waits(nc)


def _get_nc(has_bias):
    if has_bias not in _nc_cache:
        _nc_cache[has_bias] = _build(has_bias)
    return _nc_cache[has_bias]


def _prep_in_maps(x, W_qkv, b_qkv, W_proj, b_proj, pos_h, pos_w):
    import ml_dtypes

    bf16 = ml_dtypes.bfloat16
    has_bias = bool(np.any(b_qkv)) or bool(np.any(b_proj))

    inv = 1.0 / _ROPE_BASE ** (
        np.arange(0, _DR, 2, dtype=np.float32) / float(_DR)
    )  # [16]

    def rope_maps(pos):
        ang = pos.astype(np.float32)[..., None] * inv  # [B, N, 16]
        cos = np.repeat(np.cos(ang), 2, axis=-1)  # [B, N, 32]
        sin = np.repeat(np.sin(ang), 2, axis=-1)
        return cos.transpose(0, 2, 1), sin.transpose(0, 2, 1)  # [B, 32, N]

    ch, sh = rope_maps(np.asarray(pos_h))
    cw, sw = rope_maps(np.asarray(pos_w))
    cos64 = np.concatenate([ch, cw], axis=1)  # [B, 64, N]
    sin64 = np.concatenate([sh, sw], axis=1)
    sign = np.where((np.arange(64) % 32) < 16, -1.0, 1.0).astype(np.float32)
    ssin64 = sin64 * sign[None, :, None]
    cosm = np.tile(cos64, (1, 2, 1)).astype(bf16)  # [B, 128, N]
    ssinm = np.tile(ssin64, (1, 2, 1)).astype(bf16)

    xb = np.asarray(x).astype(bf16)
    xtb = xb.transpose(0, 2, 1)  # [B, C, N] feature-major per item
    wqb = np.ascontiguousarray(np.asarray(W_qkv).astype(bf16))
    wpb = np.ascontiguousarray(np.asarray(W_proj).astype(bf16))

    in_maps = []
    for i in range(_NCORES):
        lo, hi = i * _BL, (i + 1) * _BL
        m = {
            "xt": np.ascontiguousarray(xtb[lo:hi].reshape(_BL * _C, _N)),
            "wq": wqb,
            "wp": wpb,
            "cosm": np.ascontiguousarray(cosm[lo:hi].reshape(_BL * 128, _N)),
            "ssinm": np.ascontiguousarray(ssinm[lo:hi].reshape(_BL * 128, _N)),
        }
        if has_bias:
            bq = np.asarray(b_qkv).astype(bf16)
            m["bqk"] = np.ascontiguousarray(bq[:2 * _C].reshape(1, 2 * _C))
            m["bv"] = np.ascontiguousarray(bq[2 * _C:].reshape(1, _C))
            m["bp"] = np.ascontiguousarray(
                np.asarray(b_proj).astype(bf16).reshape(1, _C)
            )
        in_maps.append(m)
    return in_maps, has_bias


def _ensure_ntff_hook():
    """This image's antenv lacks axon_hooks; recreate it from the boot
    helper so run_bass_kernel_spmd(trace=True) can capture NTFF profiles."""
    import sys
    import types

    if "antenv.axon_hooks" in sys.modules:
        return
    try:
        from trn_agent_boot.trn_boot import _ntff_profile_via_ctypes

        hook = _ntff_profile_via_ctypes("/opt/axon/libaxon_pjrt.so")
    except Exception:
        hook = None
    mod = types.ModuleType("antenv.axon_hooks")
    mod._hook = hook
    mod.get_axon_ntff_profile_hook = lambda: mod._hook
    mod.set_axon_ntff_profile_hook = lambda h: setattr(mod, "_hook", h)
    sys.modules["antenv.axon_hooks"] = mod


def run(x, W_qkv, b_qkv, W_proj, b_proj, pos_h, pos_w, num_heads, **run_kwargs):
    """Build + execute on 8 NeuronCores; returns (output, BassKernelResults)."""
    from concourse.bass_utils import run_bass_kernel_spmd

    if run_kwargs.get("trace"):
        _ensure_ntff_hook()

    assert int(num_heads) == _H
    in_maps, has_bias = _prep_in_maps(
        x, W_qkv, b_qkv, W_proj, b_proj, pos_h, pos_w
    )
    nc = _get_nc(has_bias)
    res = run_bass_kernel_spmd(
        nc, in_maps, core_ids=list(range(_NCORES)), **run_kwargs
    )
    out = np.concatenate(
        [res.results[i]["out"].reshape(_BL, _N, _C) for i in range(_NCORES)],
        axis=0,
    ).astype(np.float32)
    return out, res


def kernel(x, W_qkv, b_qkv, W_proj, b_proj, pos_h, pos_w, num_heads):
    out, _ = run(x, W_qkv, b_qkv, W_proj, b_proj, pos_h, pos_w, num_heads)
    return out


# revision 20
# speedup vs baseline: 1.1329x; 1.0158x over previous
"""Trainium2 Bass kernel for nn_Attention_24704651887034.

Dense ViT-style attention block (B=16, N=1024, C=768, H=12 heads, 2D RoPE),
data-parallel over batch across 8 NeuronCores (2 batch items per core, no
collectives).

Per-core device kernel (all matmuls bf16, fp32 PSUM accumulation). All PSUM
accumulation groups run through [128,1024] 2-bank tiles so one stationary
load covers 1024 moving columns (walrus runs with ldw-opt disabled, so every
stationary switch costs ~110ns serially). Softmax normalization broadcasts
1/denom rows across partitions with a ones-outer-product matmul into PSUM
(a DMA row-broadcast costs ~11us; the matmul costs ~0.4us).

  Phase A: qk feature-major matmul + fused 2D RoPE and v token-major (with
           per-head [v_h | ones] column so attnv PSUM row 64 accumulates
           softmax denominators) for BOTH items; PSUM->SBUF copies on the
           otherwise-idle Scalar/ACT engine.
  Phase B: attention item0 (per head: sT=k'q'^T, Exp on ACT, outT=[v|1]pT),
           PSUM drains on DVE; per-head denominator row gathered to sums0.
  Phase C: attention item1, interleaved with item0's normalize (reciprocal
           on DVE + matmul row-broadcast + in-place ao multiply).
  Phase D: proj item0, item1 normalize, proj item1; proj copies on ACT.
"""

import numpy as np

_B, _N, _C, _H = 16, 1024, 768, 12
_HD, _DR = 64, 32
_ROPE_BASE = 10000.0
_NCORES = 8
_BL = _B // _NCORES  # batch items per core

_NC6 = _C // 128      # 6 contraction chunks
_NF12 = 2 * _C // 128  # 12 feature chunks for q,k

_nc_cache = {}


def _split_excess_waits(nc, max_waits=1):
    """Walrus in this toolchain accepts at most one sync-wait command per
    instruction; Tile's tail drain (and occasionally the scheduler) emits
    more. Split the excess onto same-engine NOPs inserted just before."""
    from concourse import mybir

    for f in nc.m.functions:
        for blk in f.blocks:
            insts = blk.instructions
            i = 0
            while i < len(insts):
                ins = insts[i]
                si = getattr(ins, "sync_info", None)
                if si is not None and len(si.on_wait) > max_waits:
                    excess = si.on_wait[max_waits:]
                    ins.sync_info = mybir.SyncInfo(
                        on_wait=list(si.on_wait[:max_waits]),
                        on_update=list(si.on_update),
                    )
                    for j, w in enumerate(excess):
                        nop = mybir.InstNoOp(
                            name=f"{ins.name}-sw{j}", engine=ins.engine
                        )
                        nop.sync_info = mybir.SyncInfo(on_wait=[w], on_update=[])
                        insts.insert(i, nop)
                        i += 1
                i += 1
    return nc


def _build(has_bias):
    from contextlib import ExitStack

    import concourse.bass as bass
    import concourse.tile as tile
    from concourse import mybir

    BF = mybir.dt.bfloat16
    F32 = mybir.dt.float32
    Exp = mybir.ActivationFunctionType.Exp
    N, C, H = _N, _C, _H
    BL = _BL

    nc = bass.Bass("TRN2", target_bir_lowering=False, debug=False)
    x_d = nc.dram_tensor("xt", [BL * C, N], BF, kind="ExternalInput").ap()
    wq_d = nc.dram_tensor("wq", [C, 3 * C], BF, kind="ExternalInput").ap()
    wp_d = nc.dram_tensor("wp", [C, C], BF, kind="ExternalInput").ap()
    cos_d = nc.dram_tensor("cosm", [BL * 128, N], BF, kind="ExternalInput").ap()
    sin_d = nc.dram_tensor("ssinm", [BL * 128, N], BF, kind="ExternalInput").ap()
    if has_bias:
        bqk_d = nc.dram_tensor("bqk", [1, 2 * C], BF, kind="ExternalInput").ap()
        bv_d = nc.dram_tensor("bv", [1, C], BF, kind="ExternalInput").ap()
        bp_d = nc.dram_tensor("bp", [1, C], BF, kind="ExternalInput").ap()
    out_d = nc.dram_tensor("out", [BL * N, C], BF, kind="ExternalOutput").ap()

    SH_MASK = [(i + 16) % 32 for i in range(32)]

    with tile.TileContext(nc) as tc, ExitStack() as ctx:
        const = ctx.enter_context(tc.tile_pool(name="const", bufs=1))
        xT_p = ctx.enter_context(tc.tile_pool(name="xT", bufs=12))
        map_p = ctx.enter_context(tc.tile_pool(name="maps", bufs=2))
        raw_p = ctx.enter_context(tc.tile_pool(name="qkraw", bufs=3))
        tmp_p = ctx.enter_context(tc.tile_pool(name="tmp", bufs=4))
        qkr_p = ctx.enter_context(tc.tile_pool(name="qkr", bufs=24))
        v_p = ctx.enter_context(tc.tile_pool(name="v", bufs=16))
        pT_p = ctx.enter_context(tc.tile_pool(name="pT", bufs=3))
        ao_p = ctx.enter_context(tc.tile_pool(name="ao", bufs=12))
        sum_p = ctx.enter_context(tc.tile_pool(name="sums", bufs=2))
        ost_p = ctx.enter_context(tc.tile_pool(name="ost", bufs=2))
        sc_ps = ctx.enter_context(tc.tile_pool(name="scps", bufs=2, space="PSUM"))
        bc_ps = ctx.enter_context(tc.tile_pool(name="bcps", bufs=1, space="PSUM"))
        o_ps = ctx.enter_context(tc.tile_pool(name="ops", bufs=2, space="PSUM"))

        # ---- resident constants ----
        ones64 = const.tile([1, 64], BF, tag="ones64", name="ones64")
        nc.gpsimd.memset(ones64[:], 1.0)
        wq_t = []
        for c in range(_NC6):
            t = const.tile([128, 3 * C], BF, tag=f"wq{c}", name=f"wq{c}")
            nc.sync.dma_start(
                t[:, 0:2 * C], wq_d[c * 128:(c + 1) * 128, 0:2 * C]
            )
            wq_t.append(t)
        if has_bias:
            bqk_sb = const.tile([1, 2 * C], BF, tag="bqk")
            nc.sync.dma_start(bqk_sb[:], bqk_d[:])
            bv_sb = const.tile([1, C], BF, tag="bv")
            nc.sync.dma_start(bv_sb[:], bv_d[:])
            bp_sb = const.tile([1, C], BF, tag="bp")
            nc.sync.dma_start(bp_sb[:], bp_d[:])
            ones_r = const.tile([1, 512], BF, tag="ones")
            nc.gpsimd.memset(ones_r[:], 1.0)

        # ---- inputs for both items up front (DMA overlaps compute) ----
        xT = []
        cosm = []
        ssin = []
        for b in range(BL):
            xTb = []
            for c in range(_NC6):
                t = xT_p.tile([128, N], BF, tag="xT", name=f"xT{b}_{c}")
                nc.sync.dma_start(
                    t[:], x_d[b * C + c * 128:b * C + (c + 1) * 128, :]
                )
                xTb.append(t)
            xT.append(xTb)
            cm = map_p.tile([128, N], BF, tag="cos", name=f"cos{b}")
            nc.sync.dma_start(cm[:], cos_d[b * 128:(b + 1) * 128, :])
            cosm.append(cm)
            sm = map_p.tile([128, N], BF, tag="sin", name=f"sin{b}")
            nc.sync.dma_start(sm[:], sin_d[b * 128:(b + 1) * 128, :])
            ssin.append(sm)

        # v-columns of wq and wp arrive after the hot qk inputs
        for c in range(_NC6):
            nc.sync.dma_start(
                wq_t[c][:, 2 * C:3 * C], wq_d[c * 128:(c + 1) * 128, 2 * C:3 * C]
            )
        wp_t = []
        for c in range(_NC6):
            t = const.tile([128, C], BF, tag=f"wp{c}", name=f"wp{c}")
            nc.sync.dma_start(t[:], wp_d[c * 128:(c + 1) * 128, :])
            wp_t.append(t)

        _grp = [0]

        def acc_tile(name):
            """Rotate qkv accumulation groups across 3 PSUM tiles (sc ring
            of 2 plus the broadcast bank) for 3-deep group pipelining."""
            _grp[0] += 1
            if _grp[0] % 3 == 0:
                return bc_ps.tile([128, N], F32, tag="bcp", name=name)
            return sc_ps.tile([128, N], F32, tag="sc", name=name)

        def emit_qk_feature(b, f):
            """q/k feature chunk: one stationary load per contraction chunk
            covers both 512-col halves of a 2-bank PSUM tile; fused RoPE."""
            raw = raw_p.tile([128, N], BF, tag="qkraw", name=f"raw{b}_{f}")
            ps = acc_tile(f"mq{b}_{f}")
            for c in range(_NC6):
                for t2 in range(2):
                    nc.tensor.matmul(
                        ps[:, t2 * 512:(t2 + 1) * 512],
                        wq_t[c][:, f * 128:(f + 1) * 128],
                        xT[b][c][:, t2 * 512:(t2 + 1) * 512],
                        start=(c == 0),
                        stop=(c == _NC6 - 1 and not has_bias),
                    )
            if has_bias:
                for t2 in range(2):
                    nc.tensor.matmul(
                        ps[:, t2 * 512:(t2 + 1) * 512],
                        bqk_sb[:, f * 128:(f + 1) * 128],
                        ones_r[:],
                        start=False,
                        stop=True,
                    )
            nc.scalar.copy(raw[:], ps[:])
            r = tmp_p.tile([128, N], BF, tag="ttmp", name=f"r{b}_{f}")
            nc.vector.stream_shuffle(r[:], raw[:], SH_MASK)
            tm = tmp_p.tile([128, N], BF, tag="ttmp", name=f"tm{b}_{f}")
            nc.vector.tensor_mul(tm[:], r[:], ssin[b][:])
            am = tmp_p.tile([128, N], BF, tag="ttmp", name=f"am{b}_{f}")
            nc.gpsimd.tensor_mul(am[:], raw[:], cosm[b][:])
            ro = qkr_p.tile([128, N], BF, tag="qkr", name=f"qkr{b}_{f}")
            nc.vector.tensor_add(ro[:], tm[:], am[:])
            return ro

        def emit_v(b, t8):
            """v token-major, per-head [v_h | ones] layout; one stationary
            (x chunk) load per contraction chunk covers all 768 v columns."""
            vt = v_p.tile([128, H * 65], BF, tag="v", name=f"v{b}_{t8}")
            vt3 = vt.rearrange("p (h w) -> p h w", w=65)
            nc.gpsimd.memset(vt3[:, :, 64:65], 1.0)
            ps = acc_tile(f"mv{b}_{t8}")
            for c in range(_NC6):
                for f0, fw in ((0, 512), (512, 256)):
                    nc.tensor.matmul(
                        ps[:, f0:f0 + fw],
                        xT[b][c][:, t8 * 128:(t8 + 1) * 128],
                        wq_t[c][:, 2 * C + f0:2 * C + f0 + fw],
                        start=(c == 0),
                        stop=(c == _NC6 - 1 and not has_bias),
                    )
            if has_bias:
                for f0, fw in ((0, 512), (512, 256)):
                    nc.tensor.matmul(
                        ps[:, f0:f0 + fw],
                        ones_r[:, 0:128],
                        bv_sb[:, f0:f0 + fw],
                        start=False,
                        stop=True,
                    )
            nc.scalar.copy(
                vt3[:, :, 0:64],
                ps[:, 0:768].rearrange("p (h w) -> p h w", w=64),
            )
            return vt

        def emit_head(b, h, qk_r, v_sb, ao_t, sums_st):
            """One attention head: scores -> exp -> attnv, k-chunk streamed.
            Denominator row (PSUM row 64) and ao rows drain on DVE."""
            jj, half = h // 2, (h % 2) * 64
            qh = qk_r[jj]
            kh = qk_r[6 + jj]
            op = [
                o_ps.tile([65, 512], F32, tag="o", name=f"op{b}_{h}_{i}")
                for i in range(2)
            ]
            pT_tiles = [None] * 8
            for kc in range(9):
                if kc < 8:
                    pT = pT_p.tile([128, N], BF, tag="pT", name=f"p{b}_{h}_{kc}")
                    if b == 0 or h >= 7:
                        s = acc_tile(f"s{b}_{h}_{kc}")
                    else:
                        s = sc_ps.tile(
                            [128, N], F32, tag="sc", name=f"s{b}_{h}_{kc}"
                        )
                    for qc in range(2):
                        nc.tensor.matmul(
                            s[:, qc * 512:(qc + 1) * 512],
                            kh[half:half + 64, kc * 128:(kc + 1) * 128],
                            qh[half:half + 64, qc * 512:(qc + 1) * 512],
                            start=True,
                            stop=True,
                        )
                    nc.scalar.activation(pT[:], s[:], Exp, scale=0.125)
                    pT_tiles[kc] = pT
                if kc >= 1:
                    kd = kc - 1
                    for qc in range(2):
                        nc.tensor.matmul(
                            op[qc][:],
                            v_sb[kd][:, h * 65:(h + 1) * 65],
                            pT_tiles[kd][:, qc * 512:(qc + 1) * 512],
                            start=(kd == 0),
                            stop=(kd == 7),
                        )
            srow = sum_p.tile([1, N], BF, tag="srow", bufs=1, name=f"sr{b}_{h}")
            for qc in range(2):
                nc.vector.tensor_copy(
                    srow[:, qc * 512:(qc + 1) * 512], op[qc][64:65, :]
                )
                nc.vector.tensor_copy(
                    ao_t[jj][half:half + 64, qc * 512:(qc + 1) * 512],
                    op[qc][0:64, :],
                )
            nc.gpsimd.dma_start(
                sums_st[8 * h:8 * (h + 1), :],
                srow[:].rearrange("p (a n) -> p a n", a=8),
            )

        def emit_recip(b, sums_st):
            """1/denominators for all 12 heads; each row is DMA'd back to a
            partition-0 tile so the broadcast matmul can use it as a moving
            operand (engine/matmul partition bases must be 0/32/64/96)."""
            sumf = sum_p.tile(
                [8 * H, N // 8], F32, tag="sumf", bufs=1, name=f"sf{b}"
            )
            nc.vector.tensor_copy(sumf[:], sums_st[:])
            recipf = sum_p.tile(
                [8 * H, N // 8], F32, tag="recipf", bufs=1, name=f"rf{b}"
            )
            nc.vector.reciprocal(recipf[:], sumf[:])
            recipb = sum_p.tile(
                [8 * H, N // 8], BF, tag="recipb", bufs=1, name=f"rb{b}"
            )
            nc.vector.tensor_copy(recipb[:], recipf[:])
            rows = []
            for h in range(H):
                rr = sum_p.tile(
                    [1, N], BF, tag="rbrow", bufs=4, name=f"rr{b}_{h}"
                )
                nc.gpsimd.dma_start(
                    rr[:].rearrange("p (a n) -> p a n", a=8),
                    recipb[8 * h:8 * (h + 1), :].rearrange(
                        "p (u n) -> p u n", u=1
                    ),
                )
                rows.append(rr)
            return rows

        def emit_norm(b, jj, recipb, ao_t):
            """Broadcast 1/denom rows for heads 2jj,2jj+1 across 64
            partitions each via ones-outer-product matmuls, then normalize
            the ao tile in place."""
            ps = bc_ps.tile([128, N], F32, tag="bcp", name=f"bcp{b}_{jj}")
            for k in range(2):
                for qc in range(2):
                    nc.tensor.matmul(
                        ps[k * 64:(k + 1) * 64, qc * 512:(qc + 1) * 512],
                        ones64[:],
                        recipb[2 * jj + k][:, qc * 512:(qc + 1) * 512],
                        start=True,
                        stop=True,
                    )
            nc.vector.tensor_mul(ao_t[jj][:], ao_t[jj][:], ps[:])

        def emit_proj(b, t8, ao_t, on_vec):
            """out[t8] = ao @ W_proj; one stationary (ao chunk) load per
            contraction chunk covers both 384-col output halves."""
            copy = nc.vector.tensor_copy if on_vec else nc.scalar.copy
            ot = ost_p.tile([128, C], BF, tag="ost", name=f"ost{b}_{t8}")
            ps = sc_ps.tile([128, N], F32, tag="sc", name=f"mp{b}_{t8}")
            for c in range(_NC6):
                for nf in range(2):
                    nc.tensor.matmul(
                        ps[:, nf * 512:nf * 512 + 384],
                        ao_t[c][:, t8 * 128:(t8 + 1) * 128],
                        wp_t[c][:, nf * 384:(nf + 1) * 384],
                        start=(c == 0),
                        stop=(c == _NC6 - 1 and not has_bias),
                    )
            if has_bias:
                for nf in range(2):
                    nc.tensor.matmul(
                        ps[:, nf * 512:nf * 512 + 384],
                        ones_r[:, 0:128],
                        bp_sb[:, nf * 384:(nf + 1) * 384],
                        start=False,
                        stop=True,
                    )
            for nf in range(2):
                copy(ot[:, nf * 384:(nf + 1) * 384],
                     ps[:, nf * 512:nf * 512 + 384])
            nc.sync.dma_start(
                out_d[b * N + t8 * 128:b * N + (t8 + 1) * 128, :], ot[:]
            )

        # ---- Phase A: qkv for both items (copies on idle ACT) ----
        qk_r = [[emit_qk_feature(b, f) for f in range(_NF12)]
                for b in range(BL)]
        v_sb = [[emit_v(b, t8) for t8 in range(8)] for b in range(BL)]

        # ---- Phase B: attention item0 ----
        sums0 = sum_p.tile([8 * H, N // 8], BF, tag="sums", name="sums0")
        ao0 = [
            ao_p.tile([128, N], BF, tag="ao", name=f"ao0_{i}")
            for i in range(_NC6)
        ]
        for h in range(H):
            emit_head(0, h, qk_r[0], v_sb[0], ao0, sums0)
        recipb0 = emit_recip(0, sums0)

        # ---- Phase C: attention item1, item0 normalize interleaved ----
        sums1 = sum_p.tile([8 * H, N // 8], BF, tag="sums", name="sums1")
        ao1 = [
            ao_p.tile([128, N], BF, tag="ao", name=f"ao1_{i}")
            for i in range(_NC6)
        ]
        for h in range(H):
            emit_head(1, h, qk_r[1], v_sb[1], ao1, sums1)
            if 1 <= h <= 6:
                emit_norm(0, h - 1, recipb0, ao0)
        recipb1 = emit_recip(1, sums1)

        # ---- Phase D: proj item0 (dense, on sc ring) with item1's
        # normalize interleaved on the bc bank, then proj item1 ----
        for t8 in range(8):
            emit_proj(0, t8, ao0, on_vec=False)
            if 2 <= t8 < 8 and t8 - 2 < _NC6:
                emit_norm(1, t8 - 2, recipb1, ao1)
        for t8 in range(8):
            emit_proj(1, t8, ao1, on_vec=(t8 % 2 == 1))
    return _split_excess_waits(nc)


def _get_nc(has_bias):
    if has_bias not in _nc_cache:
        _nc_cache[has_bias] = _build(has_bias)
    return _nc_cache[has_bias]


def _prep_in_maps(x, W_qkv, b_qkv, W_proj, b_proj, pos_h, pos_w):
    import ml_dtypes

    bf16 = ml_dtypes.bfloat16
    has_bias = bool(np.any(b_qkv)) or bool(np.any(b_proj))

    inv = 1.0 / _ROPE_BASE ** (
        np.arange(0, _DR, 2, dtype=np.float32) / float(_DR)
    )  # [16]

    def rope_maps(pos):
        ang = pos.astype(np.float32)[..., None] * inv  # [B, N, 16]
        cos = np.repeat(np.cos(ang), 2, axis=-1)  # [B, N, 32]
        sin = np.repeat(np.sin(ang), 2, axis=-1)
        return cos.transpose(0, 2, 1), sin.transpose(0, 2, 1)  # [B, 32, N]

    ch, sh = rope_maps(np.asarray(pos_h))
    cw, sw = rope_maps(np.asarray(pos_w))
    cos64 = np.concatenate([ch, cw], axis=1)  # [B, 64, N]
    sin64 = np.concatenate([sh, sw], axis=1)
    sign = np.where((np.arange(64) % 32) < 16, -1.0, 1.0).astype(np.float32)
    ssin64 = sin64 * sign[None, :, None]
    cosm = np.tile(cos64, (1, 2, 1)).astype(bf16)  # [B, 128, N]
    ssinm = np.tile(ssin64, (1, 2, 1)).astype(bf16)

    xb = np.asarray(x).astype(bf16)
    xtb = xb.transpose(0, 2, 1)  # [B, C, N] feature-major per item
    wqb = np.ascontiguousarray(np.asarray(W_qkv).astype(bf16))
    wpb = np.ascontiguousarray(np.asarray(W_proj).astype(bf16))

    in_maps = []
    for i in range(_NCORES):
        lo, hi = i * _BL, (i + 1) * _BL
        m = {
            "xt": np.ascontiguousarray(xtb[lo:hi].reshape(_BL * _C, _N)),
            "wq": wqb,
            "wp": wpb,
            "cosm": np.ascontiguousarray(cosm[lo:hi].reshape(_BL * 128, _N)),
            "ssinm": np.ascontiguousarray(ssinm[lo:hi].reshape(_BL * 128, _N)),
        }
        if has_bias:
            bq = np.asarray(b_qkv).astype(bf16)
            m["bqk"] = np.ascontiguousarray(bq[:2 * _C].reshape(1, 2 * _C))
            m["bv"] = np.ascontiguousarray(bq[2 * _C:].reshape(1, _C))
            m["bp"] = np.ascontiguousarray(
                np.asarray(b_proj).astype(bf16).reshape(1, _C)
            )
        in_maps.append(m)
    return in_maps, has_bias


def _ensure_ntff_hook():
    """This image's antenv lacks axon_hooks; recreate it from the boot
    helper so run_bass_kernel_spmd(trace=True) can capture NTFF profiles."""
    import sys
    import types

    if "antenv.axon_hooks" in sys.modules:
        return
    try:
        from trn_agent_boot.trn_boot import _ntff_profile_via_ctypes

        hook = _ntff_profile_via_ctypes("/opt/axon/libaxon_pjrt.so")
    except Exception:
        hook = None
    mod = types.ModuleType("antenv.axon_hooks")
    mod._hook = hook
    mod.get_axon_ntff_profile_hook = lambda: mod._hook
    mod.set_axon_ntff_profile_hook = lambda h: setattr(mod, "_hook", h)
    sys.modules["antenv.axon_hooks"] = mod


def run(x, W_qkv, b_qkv, W_proj, b_proj, pos_h, pos_w, num_heads, **run_kwargs):
    """Build + execute on 8 NeuronCores; returns (output, BassKernelResults)."""
    from concourse.bass_utils import run_bass_kernel_spmd

    if run_kwargs.get("trace"):
        _ensure_ntff_hook()

    assert int(num_heads) == _H
    in_maps, has_bias = _prep_in_maps(
        x, W_qkv, b_qkv, W_proj, b_proj, pos_h, pos_w
    )
    nc = _get_nc(has_bias)
    res = run_bass_kernel_spmd(
        nc, in_maps, core_ids=list(range(_NCORES)), **run_kwargs
    )
    out = np.concatenate(
        [res.results[i]["out"].reshape(_BL, _N, _C) for i in range(_NCORES)],
        axis=0,
    ).astype(np.float32)
    return out, res


def kernel(x, W_qkv, b_qkv, W_proj, b_proj, pos_h, pos_w, num_heads):
    out, _ = run(x, W_qkv, b_qkv, W_proj, b_proj, pos_h, pos_w, num_heads)
    return out
